# revision 9
# baseline (speedup 1.0000x reference)
"""Trainium2 Bass kernel for nn_EmotionalEmbeddingSpace (v2).

Sharding: data-parallel over batch B=16 across 8 cores (BL=2 sequences/core).
Layout: features on partitions, tokens on the free dim, in *chunk-step*
column order: col(t', cid, b) = t'*NC + cid*BL + b.

The tanh memory recurrence contracts at ~0.45/step, so each sequence is cut
into S/C chunks of C=16 positions, each warmed up from state=0 over W=16
extra steps (approximation error ~3e-6, far below bf16 noise).  All
BL*S/C = 128 chunks advance together: serial depth drops 1024 -> 32 and each
step's matmuls are 128 columns wide.  With W == C, the warmup-step pt values
are exactly the kept pt columns shifted by BL (chunk cid warms up over chunk
cid-1's positions), so no duplicate storage is needed; chunk 0 warms up on
injected zeros, which reproduces the reference's mem_{-1} = 0 exactly.

LN per layer: y evac on ScalarE, y^2/apply on DVE, column stats via
ones-column matmuls stacked into one PSUM tile (chunk c -> partition rows
c / 32+c), row math on [NCH, *] lanes at once, mean/rstd broadcast on
GpSimd, relu+bias via tensor_scalar.  encode(x) and encode(mem) run as two
interleaved half-width streams, as do the two decode halves, so serial
row-math bubbles on one chain are filled by the other.
"""

import sys

sys.path.insert(0, "/opt/trn_rl_repo")

import numpy as np
import ml_dtypes

import concourse.bass as bass
import concourse.bacc as bacc
import concourse.mybir as mybir
import concourse.tile as tile
from concourse.bass_utils import run_bass_kernel_spmd

F32 = mybir.dt.float32
BF16 = mybir.dt.bfloat16
AF = mybir.ActivationFunctionType
ALU = mybir.AluOpType

B, S_FULL, D, H, L = 16, 1024, 768, 512, 128
NCORES = 8
LN_EPS = 1e-5
NORM_EPS = 1e-8
CREC = 16   # chunk length
WREC = 16   # warmup length (must equal CREC for the shift trick)


# ---------------------------------------------------------------- host prep

def _pack_cols(*vecs):
    cols = []
    for v in vecs:
        v = np.asarray(v, np.float32).reshape(-1, 128)
        cols.append(v.T)
    return np.ascontiguousarray(np.concatenate(cols, axis=1))


def _ln_np(x, g, b, eps=LN_EPS):
    m = x.mean(-1, keepdims=True)
    v = ((x - m) ** 2).mean(-1, keepdims=True)
    return (x - m) / np.sqrt(v + eps) * g + b


def _encode_np(t, w):
    """w["W2"]/w["b2"] are the Wvo-folded effective weights."""
    h = np.maximum(_ln_np(t @ w["W1"] + w["b1"], w["g1"], w["be1"]), 0)
    g = np.maximum(_ln_np(h @ w["W2"] + w["b2"], w["g2"], w["be2"]), 0)
    zl = _ln_np(g @ w["W3"] + w["b3"], w["g3"], w["be3"])
    e = np.maximum(_ln_np(zl @ w["W4"] + w["b4"], w["g4"], w["be4"]), 0)
    return _ln_np(e @ w["W5"] + w["b5"], w["g5"], w["be5"])


# ---------------------------------------------------------------- builder

class _KB:
    WSHAPES = dict(W1=(D, H), W2=(H, H), W3=(H, L), W4=(L, H),
                   W5=(H, L), Wd1=(L, H), Wd2=(H, H), Wd3=(H, D),
                   Wm=(D, D), Um=(D, D))

    def __init__(self, S=S_FULL, BL=B // NCORES):
        self.S, self.BL = S, BL
        self.C, self.W = CREC, WREC
        assert self.C == self.W
        self.T = self.C + self.W
        self.NC = BL * S // self.C          # chunk columns per step
        self.KEPT = BL * S                  # kept token columns
        self.CH = min(512, self.KEPT)
        self.NCH = self.KEPT // self.CH
        assert self.NCH <= 16
        self.nc = bacc.Bacc("TRN2", target_bir_lowering=False, debug=False,
                            num_devices=NCORES)
        self.vec_map = {}
        self._vec_cols = 0
        self.layer_ctr = 0

    def _reg_vec(self, name, ntiles):
        self.vec_map[name] = (self._vec_cols, ntiles)
        self._vec_cols += ntiles

    def blob_layout(self):
        entries = [("xtk", 6, self.KEPT)]
        for k, (K, M) in self.WSHAPES.items():
            entries.append((k, K // 128, M))
        entries.append(("id", 1, 128))
        entries.append(("vecs", 1, self._vec_cols))
        off = {}
        pos = 0
        for name, ntiles, M in entries:
            off[name] = (pos, ntiles, M)
            pos += ntiles * M
        return off, pos

    def declare(self):
        nc = self.nc
        for nm, n in [("b1", 4), ("g1", 4), ("be1", 4),
                      ("b2", 4), ("g2", 4), ("be2", 4),
                      ("b3", 1), ("g3", 1), ("be3", 1),
                      ("b4", 4), ("g4", 4), ("be4", 4),
                      ("b5", 1), ("g5", 1), ("be5", 1),
                      ("bd1", 4), ("gd1", 4), ("bed1", 4),
                      ("bd2", 4), ("gd2", 4), ("bed2", 4),
                      ("bd3", 6), ("bm", 6), ("z0", 1), ("lneps", 1)]:
            self._reg_vec(nm, n)
        self.blob_off, nblob = self.blob_layout()
        self.d_blob = nc.dram_tensor("blob16", [128, nblob], BF16,
                                     kind="ExternalInput")
        self.d_out = nc.dram_tensor("tok_loss", [1, self.KEPT], F32,
                                    kind="ExternalOutput")
        import os as _os
        self.dbg_on = _os.environ.get("DBG_DUMP") == "1"
        if self.dbg_on:
            self.d_dbg = nc.dram_tensor("dbg", [1, 8 * self.CH], F32,
                                        kind="ExternalOutput")

    def vcol(self, name, t=0, rows=128):
        s, n = self.vec_map[name]
        assert t < n
        return self.vecs_sb[0:rows, s + t:s + t + 1]

    # ---- helpers --------------------------------------------------------
    def sel(self, q):
        """Stationary that sums columns onto PSUM row q (out rows 0..q)."""
        return self.selq[:, 96 - q:97]

    def load_weight_tiles(self, pool, wname):
        nc = self.nc
        off, ntiles, M = self.blob_off[wname]
        tiles = []
        for k in range(ntiles):
            t = pool.tile([128, M], BF16, tag=f"w_{wname}_{k}",
                          name=f"w_{wname}_{k}")
            nc.sync.dma_start(
                t[:], self.d_blob[:, off + k * M:off + (k + 1) * M])
            tiles.append(t)
        return tiles

    # ---- balanced LN layer (generator yielding per issue quantum) -------
    def layer_q(self, sid, chs_in, w_tiles, M_out, *, bias, ln=None,
                relu=False, out_override=None, out_dtype=BF16, out_slot=0,
                cs_list=None):
        nc, CH = self.nc, self.CH
        if cs_list is None:
            cs_list = sorted(chs_in.keys())
        NC_ST = len(cs_list)
        tg = {"x": "A", "m": "B", "d": "A", "d2": "B"}[sid]
        n_k = len(chs_in[cs_list[0]])
        n_m = M_out // 128
        tp = self.tmp_pool
        outs = {}
        for ci, c in enumerate(cs_list):
            if out_override is not None:
                outs[c] = [out_override[c]]
            else:
                outs[c] = [tp.tile([128, CH], out_dtype,
                                   tag=f"o{tg}{out_slot}m{m}c{ci}",
                                   name=f"o{tg}{out_slot}m{m}c{c}")[:]
                           for m in range(n_m)]
        if ln is None:
            for m in range(n_m):
                for c in cs_list:
                    ps = self.pp.tile([128, CH], F32, tag=f"ps{tg}",
                                      name=f"ps{tg}m{m}c{c}", bufs=2)
                    for k in range(n_k):
                        nc.tensor.matmul(ps[:],
                                         w_tiles[k][:, m * 128:(m + 1) * 128],
                                         chs_in[c][k], start=(k == 0),
                                         stop=(k == n_k - 1))
                    if (m + c) % 2 == 0:
                        nc.scalar.activation(outs[c][m], ps[:],
                                             AF.Relu if relu else AF.Identity,
                                             bias=self.vcol(bias, m))
                    elif relu:
                        nc.vector.tensor_scalar(
                            outs[c][m], ps[:], self.vcol(bias, m), 0.0,
                            ALU.add, ALU.max)
                    else:
                        nc.vector.tensor_scalar_add(
                            outs[c][m], ps[:], self.vcol(bias, m))
                yield
            return outs
        g_nm, be_nm = ln
        self.layer_ctr += 1
        lid = self.layer_ctr
        stY = self.sp.tile([128, CH], F32, tag=f"stY{tg}",
                           name=f"stY{tg}{lid}")
        stS = self.sp.tile([128, CH], F32, tag=f"stS{tg}",
                           name=f"stS{tg}{lid}")
        W2 = NC_ST * CH
        # fused per-m tiles spanning all chunks of this half
        ym = [tp.tile([128, W2], BF16, tag=f"y{tg}m{m}", name=f"y{tg}{lid}m{m}")
              for m in range(n_m)]
        n_mm = n_m * NC_ST
        mm_i = 0
        ci_order = list(enumerate(cs_list))[::-1]  # widest stats MM first
        for m in range(n_m):
            sq = tp.tile([128, W2], BF16, tag=f"sq{tg}", name=f"sq{tg}{lid}m{m}")
            for ci, c in ci_order:
                ps = self.pp.tile([128, CH], F32, tag=f"ps{tg}",
                                  name=f"ps{tg}m{m}c{c}", bufs=2)
                for k in range(n_k):
                    nc.tensor.matmul(ps[:],
                                     w_tiles[k][:, m * 128:(m + 1) * 128],
                                     chs_in[c][k], start=(k == 0),
                                     stop=(k == n_k - 1))
                ysl = ym[m][:, ci * CH:(ci + 1) * CH]
                sqs = sq[:, ci * CH:(ci + 1) * CH]
                nc.scalar.activation(ysl, ps[:], AF.Identity,
                                     bias=self.vcol(bias, m))
                if m % 2 == 0:
                    nc.scalar.activation(sqs, ps[:], AF.Square,
                                         bias=self.vcol(bias, m))
                else:
                    nc.vector.tensor_mul(sqs, ysl, ysl)
                q = 32 * ci
                nc.tensor.matmul(stY[0:q + 1, :], self.sel(q), ysl,
                                 start=(mm_i == 0), stop=(mm_i == n_mm - 1),
                                 skip_group_check=(0 < mm_i < n_mm - 1))
                nc.tensor.matmul(stS[0:q + 1, :], self.sel(q),
                                 sq[:, ci * CH:(ci + 1) * CH],
                                 start=(mm_i == 0), stop=(mm_i == n_mm - 1),
                                 skip_group_check=(0 < mm_i < n_mm - 1))
                mm_i += 1
            yield
        # per-chunk row math at partition 0 into one combined row:
        # [mean c0 | mean c1 | rstd c0 | rstd c1]
        inv_f = 1.0 / M_out
        r16 = self.row_pool.tile([1, 2 * W2], BF16, tag=f"r16{tg}",
                                 name=f"r16{tg}{lid}")
        for ci, c in enumerate(cs_list):
            q = 32 * ci
            rt = self.row_pool.tile([1, CH], F32, tag=f"rt{tg}c{ci}",
                                    name=f"rt{tg}{lid}c{c}")
            mseg = r16[0:1, ci * CH:(ci + 1) * CH]
            rseg = r16[0:1, W2 + ci * CH:W2 + (ci + 1) * CH]
            nc.scalar.activation(mseg, stY[q:q + 1, :], AF.Copy,
                                 scale=inv_f)
            nc.vector.scalar_tensor_tensor(rt[:], mseg, -1.0, mseg,
                                           ALU.mult, ALU.mult)
            nc.vector.scalar_tensor_tensor(rt[:], stS[q:q + 1, :], inv_f,
                                           rt[:], ALU.mult, ALU.add)
            nc.scalar.activation(rt[:], rt[:], AF.Sqrt,
                                 bias=self.vcol("lneps", rows=1))
            with nc.allow_low_precision(reason="bf16 rstd row, 0.4% rel"):
                nc.vector.reciprocal(rseg, rt[:])
        bc = tp.tile([128, 2 * W2], BF16, tag=f"bc{tg}",
                     name=f"bc{tg}{lid}")
        nc.gpsimd.partition_broadcast(bc[:], r16[:])
        yield
        # fused apply over all chunks: out = act((y - meanb)*g*rstd_b + be)
        meanb = bc[:, 0:W2]
        rstd_b = bc[:, W2:2 * W2]
        for m in range(n_m):
            u = tp.tile([128, W2], BF16, tag=f"u{tg}", name=f"u{tg}{lid}m{m}")
            nc.vector.tensor_sub(u[:], ym[m][:], meanb)
            nc.vector.scalar_tensor_tensor(u[:], u[:], self.vcol(g_nm, m),
                                           rstd_b, ALU.mult, ALU.mult)
            for ci, c in enumerate(cs_list):
                usl = u[:, ci * CH:(ci + 1) * CH]
                if relu:
                    nc.vector.tensor_scalar(
                        outs[c][m], usl, self.vcol(be_nm, m), 0.0,
                        ALU.add, ALU.max)
                else:
                    nc.vector.tensor_scalar_add(
                        outs[c][m], usl, self.vcol(be_nm, m))
            yield
        return outs

    def encode_q(self, sid, chs_in, out_override, out_dtype, cs_list):
        h = yield from self.layer_q(sid, chs_in, self.w_sb["W1"], H,
                                    bias="b1", ln=("g1", "be1"), relu=True,
                                    out_slot=0, cs_list=cs_list)
        g = yield from self.layer_q(sid, h, self.w_sb["W2"], H, bias="b2",
                                    ln=("g2", "be2"), relu=True, out_slot=1,
                                    cs_list=cs_list)
        zl = yield from self.layer_q(sid, g, self.w_sb["W3"], L, bias="b3",
                                     ln=("g3", "be3"), out_slot=0,
                                     cs_list=cs_list)
        e = yield from self.layer_q(sid, zl, self.w_sb["W4"], H, bias="b4",
                                    ln=("g4", "be4"), relu=True, out_slot=1,
                                    cs_list=cs_list)
        yield from self.layer_q(sid, e, self.w_sb["W5"], L, bias="b5",
                                ln=("g5", "be5"), out_override=out_override,
                                out_dtype=out_dtype, cs_list=cs_list)

    # ---- recurrence (generator yielding per step) -----------------------
    def recurrence_q(self, ptk4, ptkf, memc4, um, id_sb, zcol, stpv):
        """ptk4: [p, m, C, NC] kept pt; ptkf: flat [p, 6*KEPT] view;
        memc4: [p, m, C, NC] mem output; stpv: two [p, m, NC] scratch."""
        nc = self.nc
        NC, T, W, BL, KEPT = self.NC, self.T, self.W, self.BL, self.KEPT
        import os as _os
        if _os.environ.get("SKIP_REC") == "1":
            nc.vector.memset(memc4[:, :, :, :], 0.1)
            return
        G = 3 * NC

        def dst(t, g):
            if t < W:
                return stpv[t % 2][:, 3 * g:3 * g + 3, :]
            return memc4[:, 3 * g:3 * g + 3, t - W, :]

        def src(t, k):
            if t < W:
                return stpv[t % 2][:, k, :]
            return memc4[:, k, t - W, :]

        for t in range(T):
            pss = []
            for g in range(2):
                ps = self.rps.tile([128, G], F32, tag=f"rps{g}",
                                   name=f"rps{g}t{t}", bufs=2,
                                   padded_shape=[128, 512])
                for mi in range(3):
                    m = 3 * g + mi
                    if t < W:
                        # warmup: chunk cid reads chunk cid-1's kept pt
                        # (W == C); chunk 0 gets zeros.
                        nc.tensor.matmul(
                            ps[:, mi * NC:mi * NC + BL], id_sb[:],
                            zcol[:], start=(mi == 0), stop=False,
                            skip_group_check=(mi != 0))
                        last = (t == 0 and mi == 2)
                        base = m * KEPT + t * NC
                        nc.tensor.matmul(
                            ps[:, mi * NC + BL:(mi + 1) * NC], id_sb[:],
                            ptkf[:, base:base + NC - BL],
                            start=False, stop=last,
                            skip_group_check=not last)
                    else:
                        nc.tensor.matmul(
                            ps[:, mi * NC:(mi + 1) * NC], id_sb[:],
                            ptk4[:, m, t - W, :],
                            start=(mi == 0), stop=False,
                            skip_group_check=(mi != 0))
                pss.append(ps)
            for g in range(2):
                ps = pss[g]
                if t > 0:
                    for k in range(6):
                        for mi in range(3):
                            m = 3 * g + mi
                            last = (k == 5 and mi == 2)
                            nc.tensor.matmul(
                                ps[:, mi * NC:(mi + 1) * NC],
                                um[k][:, m * 128:(m + 1) * 128],
                                src(t - 1, k),
                                start=False, stop=last,
                                skip_group_check=not last)
                psv = ps[:].rearrange("p (m n) -> p m n", m=3)
                nc.scalar.activation(dst(t, g), psv[:], AF.Tanh)
            yield

    # ---- decode + recon/trans losses (generator, over a chunk subset) ---
    def decode_q(self, sid, latx, xtk, wd, lrow, cs_list):
        """lrow: [1, 2*NS*CH] partition-0 segments, per chunk:
        [recon' | 0.3*trans'] at cols [2*ci*CH, (2*ci+2)*CH)."""
        nc, CH, NC = self.nc, self.CH, self.NC
        tg = {"d": "A", "d2": "B"}[sid]
        lat16 = {c: [latx[:, c * CH:(c + 1) * CH]] for c in cs_list}
        h1 = yield from self.layer_q(sid, lat16, wd[0], H, bias="bd1",
                                     ln=("gd1", "bed1"), relu=True,
                                     out_slot=0, cs_list=cs_list)
        h2 = yield from self.layer_q(sid, h1, wd[1], H, bias="bd2",
                                     ln=("gd2", "bed2"), relu=True,
                                     out_slot=1, cs_list=cs_list)
        NS = len(cs_list)
        stY = self.sp.tile([128, CH], F32, tag=f"stY{tg}",
                           name=f"strcY{cs_list[0]}")
        stS = self.sp.tile([128, CH], F32, tag=f"stS{tg}",
                           name=f"strcS{cs_list[0]}")
        tp = self.dec_pool
        n_mm = 6 * NS
        mm_i = 0
        for m in range(6):
            for ci, c in list(enumerate(cs_list))[::-1]:
                cs = slice(c * CH, (c + 1) * CH)
                q = 32 * ci
                ps = self.pp.tile([128, CH], F32, tag=f"ps{tg}",
                                  name=f"psd{m}c{c}", bufs=2)
                for k in range(4):
                    nc.tensor.matmul(ps[:],
                                     wd[2][k][:, m * 128:(m + 1) * 128],
                                     h2[c][k], start=(k == 0), stop=(k == 3))
                r = tp.tile([128, CH], BF16, tag=f"rdc{tg}{ci}",
                            name=f"rd{tg}{m}c{c}")
                nc.vector.scalar_tensor_tensor(
                    r[:], ps[:], self.vcol("bd3", m),
                    xtk[m][:, cs], ALU.add, ALU.subtract)
                r2 = tp.tile([128, CH], BF16, tag=f"r2c{tg}{ci}",
                             name=f"r2{tg}{m}c{c}")
                nc.vector.tensor_mul(r2[:], r[:], r[:])
                nc.tensor.matmul(stY[0:q + 1, :], self.sel(q), r2[:],
                                 start=(mm_i == 0), stop=(mm_i == n_mm - 1),
                                 skip_group_check=(0 < mm_i < n_mm - 1))
                mm_i += 1
                if m == 0:
                    # trans: dif of latx vs prev kept token.  Col layout
                    # t'*NC + cid*BL + b: prev of t'=0 is (C-1)*NC + col-BL;
                    # cid=0 takes z0.
                    dif = tp.tile([128, CH], BF16, tag=f"difc{tg}{ci}",
                                  name=f"dif{tg}{c}")
                    cst = c * CH
                    if cst >= NC:
                        nc.vector.tensor_sub(dif[:],
                                             latx[:, cst:cst + CH],
                                             latx[:, cst - NC:cst + CH - NC])
                    else:
                        nc.vector.tensor_sub(
                            dif[:, NC:CH], latx[:, NC:CH],
                            latx[:, 0:CH - NC])
                        pbase = (self.C - 1) * NC
                        nc.vector.tensor_sub(
                            dif[:, self.BL:NC], latx[:, self.BL:NC],
                            latx[:, pbase:pbase + NC - self.BL])
                        for bcol in range(self.BL):
                            nc.vector.tensor_sub(
                                dif[:, bcol:bcol + 1],
                                latx[:, bcol:bcol + 1], self.z016[:])
                    d2 = tp.tile([128, CH], BF16, tag=f"sqdc{tg}{ci}",
                                 name=f"d2{tg}{c}")
                    nc.vector.tensor_mul(d2[:], dif[:], dif[:])
                    nc.tensor.matmul(stS[0:q + 1, :], self.sel(q),
                                     d2[:], start=(ci == NS - 1),
                                     stop=(ci == 0),
                                     skip_group_check=(0 < ci < NS - 1))
            yield
        if self.dbg_on and cs_list[0] == 0:
            dt2 = self.dec_pool.tile([1, 2 * CH], F32, name="dbgt2")
            nc.vector.tensor_copy(dt2[0:1, 0:CH], stY[0:1, :])
            nc.vector.tensor_copy(dt2[0:1, CH:2 * CH], stS[0:1, :])
            nc.sync.dma_start(self.d_dbg[:, 5 * CH:7 * CH], dt2[:])
        # lrow segments: [recon' | trans'] per chunk at partition 0
        for ci in range(NS):
            q = 32 * ci
            s = 2 * ci * CH
            nc.vector.tensor_scalar(lrow[0:1, s:s + CH], stY[q:q + 1, :],
                                    1.0 / D, 10.0, ALU.mult, ALU.min)
            nc.vector.tensor_scalar(lrow[0:1, s + CH:s + 2 * CH],
                                    stS[q:q + 1, :],
                                    0.3 / L, 3.0, ALU.mult, ALU.min)
        yield

    # ---- ctx loss + combine (generator, per chunk) ----------------------
    def ctx_q(self, latx, latm, lrow, cs_list):
        nc, CH = self.nc, self.CH
        tp = self.dec_pool
        for ci, c in enumerate(cs_list):
            cs = slice(c * CH, (c + 1) * CH)
            tgc = "A" if ci == 0 else "B"
            st = self.sp.tile([128, CH], F32, tag=f"stY{tgc}",
                              name=f"stcx{c}")
            u3 = tp.tile([128, CH], BF16, tag=f"difc{tgc}0", name=f"cxc{c}")
            nc.vector.tensor_mul(u3[:], latx[:, cs], latm[:, cs])
            nc.tensor.matmul(st[0:65, :], self.sel(64), u3[:],
                             start=True, stop=False)
            u2 = tp.tile([128, CH], BF16, tag=f"r2c{tgc}0", name=f"cxb{c}")
            nc.vector.tensor_mul(u2[:], latm[:, cs], latm[:, cs])
            nc.tensor.matmul(st[0:33, :], self.sel(32), u2[:],
                             start=False, stop=False, skip_group_check=True)
            u = tp.tile([128, CH], BF16, tag=f"rdc{tgc}0", name=f"cxa{c}")
            nc.vector.tensor_mul(u[:], latx[:, cs], latx[:, cs])
            nc.tensor.matmul(st[0:1, :], self.sel(0), u[:],
                             start=False, stop=True, skip_group_check=True)
            if self.dbg_on and c == 0:
                dt_ = self.dec_pool.tile([1, 3 * CH], F32, name="dbgt")
                nc.vector.tensor_copy(dt_[0:1, 0:CH], st[0:1, :])
                nc.vector.tensor_copy(dt_[0:1, CH:2 * CH], st[32:33, :])
                nc.vector.tensor_copy(dt_[0:1, 2 * CH:3 * CH], st[64:65, :])
                nc.sync.dma_start(self.d_dbg[:, 0:3 * CH], dt_[:])
                nc.sync.dma_start(self.d_dbg[:, 3 * CH:5 * CH],
                                  lrow[0:1, 0:2 * CH])
            # rows at partition 0: rx, rm, cos, combine
            rx = self.row_pool.tile([1, CH], F32, tag=f"rt{tgc}c0",
                                    name=f"cxrx{c}")
            rm = self.row_pool.tile([1, CH], F32, tag=f"rt{tgc}c1",
                                    name=f"cxrm{c}")
            nc.scalar.activation(rx[:], st[0:1, :], AF.Sqrt)
            nc.scalar.activation(rm[:], st[32:33, :], AF.Sqrt)
            nc.vector.tensor_scalar_max(rx[:], rx[:], NORM_EPS)
            nc.vector.tensor_scalar_max(rm[:], rm[:], NORM_EPS)
            nc.vector.reciprocal(rx[:], rx[:])
            nc.vector.reciprocal(rm[:], rm[:])
            cosr = self.row_pool.tile([1, CH], F32, tag="cosr",
                                      name=f"cosr{c}")
            nc.vector.tensor_mul(cosr[:], st[64:65, :], rx[:])
            nc.vector.tensor_mul(cosr[:], cosr[:], rm[:])
            # 0.3*clip(1-cos, 0, 10) then + recon' + trans'
            nc.vector.tensor_scalar(cosr[:], cosr[:], -0.3, 0.3,
                                    ALU.mult, ALU.add)
            nc.vector.tensor_scalar(cosr[:], cosr[:], 0.0, 3.0,
                                    ALU.max, ALU.min)
            s = 2 * ci * CH
            nc.vector.tensor_add(cosr[:], cosr[:], lrow[0:1, s:s + CH])
            nc.vector.tensor_add(cosr[:], cosr[:],
                                 lrow[0:1, s + CH:s + 2 * CH])
            nc.sync.dma_start(self.d_out[:, cs], cosr[:])
            yield

    # ---- main build -----------------------------------------------------
    def build(self):
        nc = self.nc
        CH, NCH, NC, T, W, C = (self.CH, self.NCH, self.NC, self.T,
                                self.W, self.C)
        KEPT, BL = self.KEPT, self.BL
        self.declare()
        with tile.TileContext(nc) as tc:
            with (
                tc.tile_pool(name="const", bufs=1) as const_pool,
                tc.tile_pool(name="wenc", bufs=1) as wenc_pool,
                tc.tile_pool(name="big", bufs=1) as big_pool,
                tc.tile_pool(name="tmp", bufs=1) as tmp_pool,
                tc.tile_pool(name="rows", bufs=1) as row_pool,
            ):
                self.tmp_pool, self.row_pool = tmp_pool, row_pool

                # ones at column 96: slicing [96-q:97] puts the ones at
                # column q of the slice -> column sums land on PSUM row q
                self.selq = const_pool.tile([128, 97], BF16, name="selq")
                nc.vector.memset(self.selq[:], 0.0)
                nc.vector.memset(self.selq[:, 96:97], 1.0)
                zcol = const_pool.tile([128, BL], BF16, name="zcol")
                nc.vector.memset(zcol[:], 0.0)
                voff = self.blob_off["vecs"][0]
                vecs16 = const_pool.tile([128, self._vec_cols], BF16,
                                         name="vecs16")
                nc.sync.dma_start(
                    vecs16[:], self.d_blob[:, voff:voff + self._vec_cols])
                self.vecs_sb = const_pool.tile([128, self._vec_cols], F32)
                nc.vector.tensor_copy(self.vecs_sb[:], vecs16[:])
                self.z016 = const_pool.tile([128, 1], BF16, name="z016")
                nc.vector.tensor_copy(self.z016[:], self.vcol("z0"))

                self.w_sb = {}
                for k in ("W1", "W2", "W3", "W4", "W5"):
                    self.w_sb[k] = self.load_weight_tiles(wenc_pool, k)

                latx = big_pool.tile([128, KEPT], BF16, tag="latx",
                                     name="latx")
                latm = big_pool.tile([128, KEPT], BF16, tag="latm",
                                     name="latm")
                memw = big_pool.tile([128, 6 * KEPT], BF16, tag="memw",
                                     name="memw")

                # xtk: kept tokens in chunk-step order
                xtk_cm = tc.tile_pool(name="xtkp", bufs=1)
                xtk_pool = xtk_cm.__enter__()
                xtk = [xtk_pool.tile([128, KEPT], BF16, tag=f"xtk{k}",
                                     name=f"xtk{k}") for k in range(6)]
                off, _, M = self.blob_off["xtk"]
                for k in range(6):
                    nc.sync.dma_start(
                        xtk[k][:],
                        self.d_blob[:, off + k * M:off + (k + 1) * M])

                # ==== phase 0: ptk = Wm^T xtk + bm (kept cols only)
                pt_cm = tc.tile_pool(name="ptp", bufs=1)
                pt_pool = pt_cm.__enter__()
                ptw = pt_pool.tile([128, 6 * KEPT], BF16, tag="ptw",
                                   name="ptw")
                ptk4 = ptw[:].rearrange("p (m t n) -> p m t n", m=6, t=C)
                wm_cm = tc.tile_pool(name="wmp", bufs=1)
                wm_pool = wm_cm.__enter__()
                wm = self.load_weight_tiles(wm_pool, "Wm")
                with tc.tile_pool(name="ps0", bufs=1, space="PSUM") as pp0:
                    for m in range(6):
                        for base in range(0, KEPT, CH):
                            n = min(CH, KEPT - base)
                            ps = pp0.tile([128, CH], F32, tag="p0",
                                          name=f"p0m{m}b{base}", bufs=2)
                            for k in range(6):
                                nc.tensor.matmul(
                                    ps[:, 0:n],
                                    wm[k][:, m * 128:(m + 1) * 128],
                                    xtk[k][:, base:base + n],
                                    start=(k == 0), stop=(k == 5))
                            pb = m * KEPT + base
                            if m % 2 == 0:
                                nc.scalar.activation(
                                    ptw[:, pb:pb + n], ps[:, 0:n],
                                    AF.Identity, bias=self.vcol("bm", m))
                            else:
                                nc.vector.tensor_scalar_add(
                                    ptw[:, pb:pb + n], ps[:, 0:n],
                                    self.vcol("bm", m))

                wm_cm.__exit__(None, None, None)

                # ==== phase 1: recurrence (standalone)
                um_cm = tc.tile_pool(name="ump", bufs=1)
                um_pool = um_cm.__enter__()
                um = self.load_weight_tiles(um_pool, "Um")
                id_off = self.blob_off["id"][0]
                id_sb = um_pool.tile([128, 128], BF16, name="id_sb")
                nc.sync.dma_start(id_sb[:],
                                  self.d_blob[:, id_off:id_off + 128])
                stp_cm = tc.tile_pool(name="stp", bufs=1)
                stp_pool = stp_cm.__enter__()
                rps_cm = tc.tile_pool(name="recps", bufs=1, space="PSUM")
                self.rps = rps_cm.__enter__()
                stpv = [stp_pool.tile([128, 6 * NC], BF16, tag=f"stp{i}",
                                      name=f"stp{i}")[:].rearrange(
                            "p (m n) -> p m n", m=6)
                        for i in range(2)]
                memc4 = memw[:].rearrange("p (m t n) -> p m t n", m=6, t=C)
                for _ in self.recurrence_q(ptk4, ptw[:], memc4, um,
                                           id_sb, zcol, stpv):
                    pass
                rps_cm.__exit__(None, None, None)
                stp_cm.__exit__(None, None, None)
                um_cm.__exit__(None, None, None)
                pt_cm.__exit__(None, None, None)

                # long-lived MLP PSUM pools (phases 2-5)
                mlp_ps = tc.tile_pool(name="mps", bufs=1, space="PSUM")
                self.pp = mlp_ps.__enter__()
                mlp_sp = tc.tile_pool(name="msp", bufs=1, space="PSUM")
                self.sp = mlp_sp.__enter__()

                # ==== phase 2: encode(x) || encode(mem), half-width streams
                allc = list(range(NCH))
                xt_chs = {c: [xtk[k][:, c * CH:(c + 1) * CH]
                              for k in range(6)] for c in allc}
                lat_ov = {c: latx[:, c * CH:(c + 1) * CH] for c in allc}
                latm_ov = {c: latm[:, c * CH:(c + 1) * CH] for c in allc}
                memv = memw[:].rearrange("p (m tn) -> p m tn", m=6)
                mem_chs = {c: [memv[:, k, c * CH:(c + 1) * CH]
                               for k in range(6)] for c in allc}
                h0 = max(1, NCH // 2)
                import os as _os
                if _os.environ.get("SKIP_ENC") == "1":
                    nc.vector.memset(latx[:], 0.1)
                    nc.vector.memset(latm[:], 0.1)
                else:
                    g_x = _chain(
                        self.encode_q("x", xt_chs, lat_ov, BF16, allc[:h0]),
                        self.encode_q("x", xt_chs, lat_ov, BF16, allc[h0:])
                        if NCH > 1 else None)
                    g_m = _chain(
                        self.encode_q("m", mem_chs, latm_ov, BF16,
                                      allc[:h0]),
                        self.encode_q("m", mem_chs, latm_ov, BF16,
                                      allc[h0:])
                        if NCH > 1 else None)
                    _interleave([g_x, g_m], [1, 1])

                # ==== phase 3: decode + recon/trans (2 half passes)
                wdec_cm = tc.tile_pool(name="wdec", bufs=1)
                wdec_pool = wdec_cm.__enter__()
                self.dec_pool = wdec_pool
                halves = [allc[:h0], allc[h0:]] if NCH > 1 else [allc]
                lrows = [wdec_pool.tile([1, 2 * len(csl) * CH], F32,
                                        tag=f"lrow{h}", name=f"lrow{h}")
                         for h, csl in enumerate(halves)]
                wd = [self.load_weight_tiles(wdec_pool, k)
                      for k in ("Wd1", "Wd2", "Wd3")]
                if _os.environ.get("SKIP_DEC") == "1":
                    for lr in lrows:
                        nc.vector.memset(lr[:], 0.1)
                else:
                    decs = [self.decode_q("d" if h == 0 else "d2", latx,
                                          xtk, wd, lrows[h], csl)
                            for h, csl in enumerate(halves)]
                    _interleave(decs, [1] * len(decs))

                # ==== phase 5: ctx + combine + output
                for h, csl in enumerate(halves):
                    for _ in self.ctx_q(latx, latm, lrows[h], csl):
                        pass

                wdec_cm.__exit__(None, None, None)
                mlp_sp.__exit__(None, None, None)
                mlp_ps.__exit__(None, None, None)
                xtk_cm.__exit__(None, None, None)
        nc.compile()
        return nc


def _chain(*gens):
    for g in gens:
        if g is not None:
            yield from g


def _interleave(gens, weights):
    gens = list(gens)
    weights = list(weights)
    while gens:
        for i in range(len(gens) - 1, -1, -1):
            try:
                for _ in range(weights[i]):
                    next(gens[i])
            except StopIteration:
                del gens[i]
                del weights[i]


# ---------------------------------------------------------------- runner

_CACHE = {}


def _get_built(S, BL):
    key = (S, BL)
    if key not in _CACHE:
        kb = _KB(S, BL)
        kb.build()
        _CACHE[key] = kb
    return _CACHE[key]


def _host_inputs(kb, inputs):
    S, BL, C, W, T, NC = kb.S, kb.BL, kb.C, kb.W, kb.T, kb.NC
    w = {k: np.asarray(v, np.float32) for k, v in inputs.items()}
    Wvo = w["Wv"] @ w["Wo"]
    bvo = w["bv"] @ w["Wo"] + w["bo"]
    wd = dict(w)
    # fold the (linear) self-attn projection into W2: a@W2 = h@(Wvo@W2)
    wd["W2"] = Wvo @ w["W2"]
    wd["b2"] = bvo @ w["W2"] + w["b2"]
    z0 = _encode_np(np.zeros((1, D), np.float32), wd)[0]

    vecs = _pack_cols(w["b1"], w["g1"], w["be1"],
                      wd["b2"], w["g2"], w["be2"],
                      w["b3"], w["g3"], w["be3"],
                      w["b4"], w["g4"], w["be4"],
                      w["b5"], w["g5"], w["be5"],
                      w["bd1"], w["gd1"], w["bed1"],
                      w["bd2"], w["gd2"], w["bed2"],
                      w["bd3"], w["bm"], z0,
                      np.full(128, LN_EPS, np.float32))

    def b16(x):
        return np.ascontiguousarray(x.astype(ml_dtypes.bfloat16))

    wd["id"] = np.eye(128, dtype=np.float32)
    wd["vecs"] = vecs
    blob_off, nblob = kb.blob_layout()
    wblob = np.zeros((128, nblob), ml_dtypes.bfloat16)
    for name, (off, ntiles, M) in blob_off.items():
        if name == "xtk":
            continue
        wsrc = np.asarray(wd[name], np.float32)
        for k in range(ntiles):
            wblob[:, off + k * M:off + (k + 1) * M] = b16(
                wsrc[k * 128:(k + 1) * 128, :])

    # kept tokens in chunk-step order: col(t', cid, b) = t'*NC + cid*BL + b
    seqs = np.asarray(inputs["sequences"], np.float32)
    ncid = S // C
    in_maps = []
    xtk_off = blob_off["xtk"]
    for core in range(NCORES):
        xs = seqs[core * BL:(core + 1) * BL, :S, :]       # [BL,S,D]
        g = xs.reshape(BL, ncid, C, D)                    # [BL,cid,t',D]
        g = np.transpose(g, (2, 1, 0, 3))                 # [t',cid,BL,D]
        gt16 = b16(g.reshape(kb.KEPT, D).T)               # [D, KEPT]
        blob = wblob.copy()
        off, _, M = xtk_off
        for k in range(6):
            blob[:, off + k * M:off + (k + 1) * M] = \
                gt16[k * 128:(k + 1) * 128, :]
        in_maps.append(dict(blob16=blob))
    return in_maps


def _l2_term(inputs):
    names = ["W1", "b1", "g1", "be1", "Wv", "bv", "Wo", "bo", "W2", "b2",
             "g2", "be2", "W3", "b3", "g3", "be3", "W4", "b4", "g4", "be4",
             "W5", "b5", "g5", "be5", "Wd1", "bd1", "gd1", "bed1", "Wd2",
             "bd2", "gd2", "bed2", "Wd3", "bd3", "Wm", "Um", "bm"]
    l2 = sum(np.linalg.norm(np.asarray(inputs[n], np.float64))
             for n in names)
    return float(np.clip(l2, 0.0, 10.0))


def _combine(kb, res, inputs):
    tok = np.concatenate([res.results[c]["tok_loss"].reshape(-1)
                          for c in range(NCORES)])
    l2 = _l2_term(inputs)
    per_tok = np.clip(tok.astype(np.float64) + 1e-4 * l2, 0.0, 100.0)
    nb = kb.BL * NCORES
    return np.float32(per_tok.sum() / nb)


def kernel(**inputs):
    seqs = np.asarray(inputs["sequences"])
    S = seqs.shape[1]
    BL = seqs.shape[0] // NCORES
    kb = _get_built(S, BL)
    in_maps = _host_inputs(kb, inputs)
    res = run_bass_kernel_spmd(kb.nc, in_maps, list(range(NCORES)))
    return _combine(kb, res, inputs)


# revision 10
# speedup vs baseline: 1.0068x; 1.0068x over previous
"""Trainium2 Bass kernel for nn_EmotionalEmbeddingSpace (v2).

Sharding: data-parallel over batch B=16 across 8 cores (BL=2 sequences/core).
Layout: features on partitions, tokens on the free dim, in *chunk-step*
column order: col(t', cid, b) = t'*NC + cid*BL + b.

The tanh memory recurrence contracts at ~0.45/step, so each sequence is cut
into S/C chunks of C=16 positions, each warmed up from state=0 over W=16
extra steps (approximation error ~3e-6, far below bf16 noise).  All
BL*S/C = 128 chunks advance together: serial depth drops 1024 -> 32 and each
step's matmuls are 128 columns wide.  With W == C, the warmup-step pt values
are exactly the kept pt columns shifted by BL (chunk cid warms up over chunk
cid-1's positions), so no duplicate storage is needed; chunk 0 warms up on
injected zeros, which reproduces the reference's mem_{-1} = 0 exactly.

LN per layer: y evac on ScalarE, y^2/apply on DVE, column stats via
ones-column matmuls stacked into one PSUM tile (chunk c -> partition rows
c / 32+c), row math on [NCH, *] lanes at once, mean/rstd broadcast on
GpSimd, relu+bias via tensor_scalar.  encode(x) and encode(mem) run as two
interleaved half-width streams, as do the two decode halves, so serial
row-math bubbles on one chain are filled by the other.
"""

import sys

sys.path.insert(0, "/opt/trn_rl_repo")

import numpy as np
import ml_dtypes

import concourse.bass as bass
import concourse.bacc as bacc
import concourse.mybir as mybir
import concourse.tile as tile
from concourse.bass_utils import run_bass_kernel_spmd

F32 = mybir.dt.float32
BF16 = mybir.dt.bfloat16
AF = mybir.ActivationFunctionType
ALU = mybir.AluOpType

B, S_FULL, D, H, L = 16, 1024, 768, 512, 128
NCORES = 8
LN_EPS = 1e-5
NORM_EPS = 1e-8
CREC = 16   # chunk length
WREC = 16   # warmup length (must equal CREC for the shift trick)


# ---------------------------------------------------------------- host prep

def _pack_cols(*vecs):
    cols = []
    for v in vecs:
        v = np.asarray(v, np.float32).reshape(-1, 128)
        cols.append(v.T)
    return np.ascontiguousarray(np.concatenate(cols, axis=1))


def _ln_np(x, g, b, eps=LN_EPS):
    m = x.mean(-1, keepdims=True)
    v = ((x - m) ** 2).mean(-1, keepdims=True)
    return (x - m) / np.sqrt(v + eps) * g + b


def _encode_np(t, w):
    """w["W2"]/w["b2"] are the Wvo-folded effective weights."""
    h = np.maximum(_ln_np(t @ w["W1"] + w["b1"], w["g1"], w["be1"]), 0)
    g = np.maximum(_ln_np(h @ w["W2"] + w["b2"], w["g2"], w["be2"]), 0)
    zl = _ln_np(g @ w["W3"] + w["b3"], w["g3"], w["be3"])
    e = np.maximum(_ln_np(zl @ w["W4"] + w["b4"], w["g4"], w["be4"]), 0)
    return _ln_np(e @ w["W5"] + w["b5"], w["g5"], w["be5"])


# ---------------------------------------------------------------- builder

class _KB:
    WSHAPES = dict(W1=(D, H), W2=(H, H), W3=(H, L), W4=(L, H),
                   W5=(H, L), Wd1=(L, H), Wd2=(H, H), Wd3=(H, D),
                   Wm=(D, D), Um=(D, D))

    def __init__(self, S=S_FULL, BL=B // NCORES):
        self.S, self.BL = S, BL
        self.C, self.W = CREC, WREC
        assert self.C == self.W
        self.T = self.C + self.W
        self.NC = BL * S // self.C          # chunk columns per step
        self.KEPT = BL * S                  # kept token columns
        self.CH = min(512, self.KEPT)
        self.NCH = self.KEPT // self.CH
        assert self.NCH <= 16
        self.nc = bacc.Bacc("TRN2", target_bir_lowering=False, debug=False,
                            num_devices=NCORES)
        self.vec_map = {}
        self._vec_cols = 0
        self.layer_ctr = 0

    def _reg_vec(self, name, ntiles):
        self.vec_map[name] = (self._vec_cols, ntiles)
        self._vec_cols += ntiles

    def blob_layout(self):
        entries = [("xtk", 6, self.KEPT)]
        for k, (K, M) in self.WSHAPES.items():
            entries.append((k, K // 128, M))
        entries.append(("id", 1, 128))
        entries.append(("vecs", 1, self._vec_cols))
        off = {}
        pos = 0
        for name, ntiles, M in entries:
            off[name] = (pos, ntiles, M)
            pos += ntiles * M
        return off, pos

    def declare(self):
        nc = self.nc
        for nm, n in [("b1", 4), ("g1", 4), ("be1", 4),
                      ("b2", 4), ("g2", 4), ("be2", 4),
                      ("b3", 1), ("g3", 1), ("be3", 1),
                      ("b4", 4), ("g4", 4), ("be4", 4),
                      ("b5", 1), ("g5", 1), ("be5", 1),
                      ("bd1", 4), ("gd1", 4), ("bed1", 4),
                      ("bd2", 4), ("gd2", 4), ("bed2", 4),
                      ("bd3", 6), ("bm", 6), ("z0", 1), ("lneps", 1)]:
            self._reg_vec(nm, n)
        self.blob_off, nblob = self.blob_layout()
        self.d_blob = nc.dram_tensor("blob16", [128, nblob], BF16,
                                     kind="ExternalInput")
        self.d_out = nc.dram_tensor("tok_loss", [1, self.KEPT], F32,
                                    kind="ExternalOutput")
        import os as _os
        self.dbg_on = _os.environ.get("DBG_DUMP") == "1"
        if self.dbg_on:
            self.d_dbg = nc.dram_tensor("dbg", [1, 8 * self.CH], F32,
                                        kind="ExternalOutput")

    def vcol(self, name, t=0, rows=128):
        s, n = self.vec_map[name]
        assert t < n
        return self.vecs_sb[0:rows, s + t:s + t + 1]

    # ---- helpers --------------------------------------------------------
    def sel(self, q):
        """Stationary that sums columns onto PSUM row q (out rows 0..q)."""
        return self.selq[:, 96 - q:97]

    def load_weight_tiles(self, pool, wname):
        nc = self.nc
        off, ntiles, M = self.blob_off[wname]
        tiles = []
        for k in range(ntiles):
            t = pool.tile([128, M], BF16, tag=f"w_{wname}_{k}",
                          name=f"w_{wname}_{k}")
            nc.sync.dma_start(
                t[:], self.d_blob[:, off + k * M:off + (k + 1) * M])
            tiles.append(t)
        return tiles

    # ---- balanced LN layer (generator yielding per issue quantum) -------
    def layer_q(self, sid, chs_in, w_tiles, M_out, *, bias, ln=None,
                relu=False, out_override=None, out_dtype=BF16, out_slot=0,
                cs_list=None):
        nc, CH = self.nc, self.CH
        if cs_list is None:
            cs_list = sorted(chs_in.keys())
        NC_ST = len(cs_list)
        tg = {"x": "A", "m": "B", "d": "A", "d2": "B"}[sid]
        n_k = len(chs_in[cs_list[0]])
        n_m = M_out // 128
        tp = self.tmp_pool
        outs = {}
        for ci, c in enumerate(cs_list):
            if out_override is not None:
                outs[c] = [out_override[c]]
            else:
                outs[c] = [tp.tile([128, CH], out_dtype,
                                   tag=f"o{tg}{out_slot}m{m}c{ci}",
                                   name=f"o{tg}{out_slot}m{m}c{c}")[:]
                           for m in range(n_m)]
        if ln is None:
            for m in range(n_m):
                for c in cs_list:
                    ps = self.pp.tile([128, CH], F32, tag=f"ps{tg}",
                                      name=f"ps{tg}m{m}c{c}", bufs=2)
                    for k in range(n_k):
                        nc.tensor.matmul(ps[:],
                                         w_tiles[k][:, m * 128:(m + 1) * 128],
                                         chs_in[c][k], start=(k == 0),
                                         stop=(k == n_k - 1))
                    if (m + c) % 2 == 0:
                        nc.scalar.activation(outs[c][m], ps[:],
                                             AF.Relu if relu else AF.Identity,
                                             bias=self.vcol(bias, m))
                    elif relu:
                        nc.vector.tensor_scalar(
                            outs[c][m], ps[:], self.vcol(bias, m), 0.0,
                            ALU.add, ALU.max)
                    else:
                        nc.vector.tensor_scalar_add(
                            outs[c][m], ps[:], self.vcol(bias, m))
                yield
            return outs
        g_nm, be_nm = ln
        self.layer_ctr += 1
        lid = self.layer_ctr
        stY = self.sp.tile([128, CH], F32, tag=f"stY{tg}",
                           name=f"stY{tg}{lid}")
        stS = self.sp.tile([128, CH], F32, tag=f"stS{tg}",
                           name=f"stS{tg}{lid}")
        W2 = NC_ST * CH
        # fused per-m tiles spanning all chunks of this half
        ym = [tp.tile([128, W2], BF16, tag=f"y{tg}m{m}", name=f"y{tg}{lid}m{m}")
              for m in range(n_m)]
        n_mm = n_m * NC_ST
        mm_i = 0
        ci_order = list(enumerate(cs_list))[::-1]  # widest stats MM first
        for m in range(n_m):
            sq = tp.tile([128, W2], BF16, tag=f"sq{tg}", name=f"sq{tg}{lid}m{m}")
            for ci, c in ci_order:
                ps = self.pp.tile([128, CH], F32, tag=f"ps{tg}",
                                  name=f"ps{tg}m{m}c{c}", bufs=2)
                for k in range(n_k):
                    nc.tensor.matmul(ps[:],
                                     w_tiles[k][:, m * 128:(m + 1) * 128],
                                     chs_in[c][k], start=(k == 0),
                                     stop=(k == n_k - 1))
                ysl = ym[m][:, ci * CH:(ci + 1) * CH]
                sqs = sq[:, ci * CH:(ci + 1) * CH]
                nc.scalar.activation(ysl, ps[:], AF.Identity,
                                     bias=self.vcol(bias, m))
                if m % 2 == 0:
                    nc.scalar.activation(sqs, ps[:], AF.Square,
                                         bias=self.vcol(bias, m))
                else:
                    nc.vector.tensor_mul(sqs, ysl, ysl)
                q = 32 * ci
                nc.tensor.matmul(stY[0:q + 1, :], self.sel(q), ysl,
                                 start=(mm_i == 0), stop=(mm_i == n_mm - 1),
                                 skip_group_check=(0 < mm_i < n_mm - 1))
                nc.tensor.matmul(stS[0:q + 1, :], self.sel(q),
                                 sq[:, ci * CH:(ci + 1) * CH],
                                 start=(mm_i == 0), stop=(mm_i == n_mm - 1),
                                 skip_group_check=(0 < mm_i < n_mm - 1))
                mm_i += 1
            yield
        # per-chunk row math at partition 0 into one combined row:
        # [mean c0 | mean c1 | rstd c0 | rstd c1]
        inv_f = 1.0 / M_out
        r16 = self.row_pool.tile([1, 2 * W2], BF16, tag=f"r16{tg}",
                                 name=f"r16{tg}{lid}")
        for ci, c in enumerate(cs_list):
            q = 32 * ci
            rt = self.row_pool.tile([1, CH], F32, tag=f"rt{tg}c{ci}",
                                    name=f"rt{tg}{lid}c{c}")
            mseg = r16[0:1, ci * CH:(ci + 1) * CH]
            rseg = r16[0:1, W2 + ci * CH:W2 + (ci + 1) * CH]
            nc.scalar.activation(mseg, stY[q:q + 1, :], AF.Copy,
                                 scale=inv_f)
            nc.vector.scalar_tensor_tensor(rt[:], mseg, -1.0, mseg,
                                           ALU.mult, ALU.mult)
            nc.vector.scalar_tensor_tensor(rt[:], stS[q:q + 1, :], inv_f,
                                           rt[:], ALU.mult, ALU.add)
            nc.scalar.activation(rt[:], rt[:], AF.Sqrt,
                                 bias=self.vcol("lneps", rows=1))
            with nc.allow_low_precision(reason="bf16 rstd row, 0.4% rel"):
                nc.vector.reciprocal(rseg, rt[:])
        bc = tp.tile([128, 2 * W2], BF16, tag=f"bc{tg}",
                     name=f"bc{tg}{lid}")
        nc.gpsimd.partition_broadcast(bc[:], r16[:])
        yield
        # fused apply over all chunks: out = act((y - meanb)*g*rstd_b + be)
        meanb = bc[:, 0:W2]
        rstd_b = bc[:, W2:2 * W2]
        for m in range(n_m):
            u = tp.tile([128, W2], BF16, tag=f"u{tg}", name=f"u{tg}{lid}m{m}")
            nc.vector.tensor_sub(u[:], ym[m][:], meanb)
            nc.vector.scalar_tensor_tensor(u[:], u[:], self.vcol(g_nm, m),
                                           rstd_b, ALU.mult, ALU.mult)
            for ci, c in enumerate(cs_list):
                usl = u[:, ci * CH:(ci + 1) * CH]
                if relu:
                    nc.vector.tensor_scalar(
                        outs[c][m], usl, self.vcol(be_nm, m), 0.0,
                        ALU.add, ALU.max)
                else:
                    nc.vector.tensor_scalar_add(
                        outs[c][m], usl, self.vcol(be_nm, m))
            yield
        return outs

    def encode_q(self, sid, chs_in, out_override, out_dtype, cs_list):
        h = yield from self.layer_q(sid, chs_in, self.w_sb["W1"], H,
                                    bias="b1", ln=("g1", "be1"), relu=True,
                                    out_slot=0, cs_list=cs_list)
        g = yield from self.layer_q(sid, h, self.w_sb["W2"], H, bias="b2",
                                    ln=("g2", "be2"), relu=True, out_slot=1,
                                    cs_list=cs_list)
        zl = yield from self.layer_q(sid, g, self.w_sb["W3"], L, bias="b3",
                                     ln=("g3", "be3"), out_slot=0,
                                     cs_list=cs_list)
        e = yield from self.layer_q(sid, zl, self.w_sb["W4"], H, bias="b4",
                                    ln=("g4", "be4"), relu=True, out_slot=1,
                                    cs_list=cs_list)
        yield from self.layer_q(sid, e, self.w_sb["W5"], L, bias="b5",
                                ln=("g5", "be5"), out_override=out_override,
                                out_dtype=out_dtype, cs_list=cs_list)

    # ---- recurrence (generator yielding per step) -----------------------
    def recurrence_q(self, ptk4, ptkf, memc4, um, id_sb, zcol, stpv):
        """ptk4: [p, m, C, NC] kept pt; ptkf: flat [p, 6*KEPT] view;
        memc4: [p, m, C, NC] mem output; stpv: two [p, m, NC] scratch."""
        nc = self.nc
        NC, T, W, BL, KEPT = self.NC, self.T, self.W, self.BL, self.KEPT
        import os as _os
        if _os.environ.get("SKIP_REC") == "1":
            nc.vector.memset(memc4[:, :, :, :], 0.1)
            return
        G = 3 * NC

        def dst(t, g):
            if t < W:
                return stpv[t % 2][:, 3 * g:3 * g + 3, :]
            return memc4[:, 3 * g:3 * g + 3, t - W, :]

        def src(t, k):
            if t < W:
                return stpv[t % 2][:, k, :]
            return memc4[:, k, t - W, :]

        for t in range(T):
            pss = []
            for g in range(2):
                ps = self.rps.tile([128, G], F32, tag=f"rps{g}",
                                   name=f"rps{g}t{t}", bufs=2,
                                   padded_shape=[128, 512])
                for mi in range(3):
                    m = 3 * g + mi
                    if t < W:
                        # warmup: chunk cid reads chunk cid-1's kept pt
                        # (W == C); chunk 0 gets zeros.
                        nc.tensor.matmul(
                            ps[:, mi * NC:mi * NC + BL], id_sb[:],
                            zcol[:], start=(mi == 0), stop=False,
                            skip_group_check=(mi != 0))
                        last = (t == 0 and mi == 2)
                        base = m * KEPT + t * NC
                        nc.tensor.matmul(
                            ps[:, mi * NC + BL:(mi + 1) * NC], id_sb[:],
                            ptkf[:, base:base + NC - BL],
                            start=False, stop=last,
                            skip_group_check=not last)
                    else:
                        nc.tensor.matmul(
                            ps[:, mi * NC:(mi + 1) * NC], id_sb[:],
                            ptk4[:, m, t - W, :],
                            start=(mi == 0), stop=False,
                            skip_group_check=(mi != 0))
                pss.append(ps)
            for g in range(2):
                ps = pss[g]
                if t > 0:
                    for k in range(6):
                        for mi in range(3):
                            m = 3 * g + mi
                            last = (k == 5 and mi == 2)
                            nc.tensor.matmul(
                                ps[:, mi * NC:(mi + 1) * NC],
                                um[k][:, m * 128:(m + 1) * 128],
                                src(t - 1, k),
                                start=False, stop=last,
                                skip_group_check=not last)
                psv = ps[:].rearrange("p (m n) -> p m n", m=3)
                nc.scalar.activation(dst(t, g), psv[:], AF.Tanh)
            yield

    # ---- decode + recon/trans losses (generator, over a chunk subset) ---
    def decode_q(self, sid, latx, xtk, wd, lrow, cs_list):
        """lrow: [1, 2*NS*CH] partition-0 segments, per chunk:
        [recon' | 0.3*trans'] at cols [2*ci*CH, (2*ci+2)*CH)."""
        nc, CH, NC = self.nc, self.CH, self.NC
        tg = {"d": "A", "d2": "B"}[sid]
        lat16 = {c: [latx[:, c * CH:(c + 1) * CH]] for c in cs_list}
        h1 = yield from self.layer_q(sid, lat16, wd[0], H, bias="bd1",
                                     ln=("gd1", "bed1"), relu=True,
                                     out_slot=0, cs_list=cs_list)
        h2 = yield from self.layer_q(sid, h1, wd[1], H, bias="bd2",
                                     ln=("gd2", "bed2"), relu=True,
                                     out_slot=1, cs_list=cs_list)
        NS = len(cs_list)
        stY = self.sp.tile([128, CH], F32, tag=f"stY{tg}",
                           name=f"strcY{cs_list[0]}")
        stS = self.sp.tile([128, CH], F32, tag=f"stS{tg}",
                           name=f"strcS{cs_list[0]}")
        tp = self.dec_pool
        n_mm = 6 * NS
        mm_i = 0
        W2r = NS * CH
        for m in range(6):
            r = tp.tile([128, W2r], BF16, tag=f"rdc{tg}",
                        name=f"rd{tg}{m}")
            r2 = tp.tile([128, W2r], BF16, tag=f"r2c{tg}",
                         name=f"r2{tg}{m}")
            for ci, c in list(enumerate(cs_list))[::-1]:
                cs = slice(c * CH, (c + 1) * CH)
                ps = self.pp.tile([128, CH], F32, tag=f"ps{tg}",
                                  name=f"psd{m}c{c}", bufs=2)
                for k in range(4):
                    nc.tensor.matmul(ps[:],
                                     wd[2][k][:, m * 128:(m + 1) * 128],
                                     h2[c][k], start=(k == 0), stop=(k == 3))
                nc.vector.scalar_tensor_tensor(
                    r[:, ci * CH:(ci + 1) * CH], ps[:], self.vcol("bd3", m),
                    xtk[m][:, cs], ALU.add, ALU.subtract)
            nc.vector.tensor_mul(r2[:], r[:], r[:])
            for ci, c in list(enumerate(cs_list))[::-1]:
                q = 32 * ci
                nc.tensor.matmul(stY[0:q + 1, :], self.sel(q),
                                 r2[:, ci * CH:(ci + 1) * CH],
                                 start=(mm_i == 0), stop=(mm_i == n_mm - 1),
                                 skip_group_check=(0 < mm_i < n_mm - 1))
                mm_i += 1
            for ci, c in list(enumerate(cs_list))[::-1]:
                cs = slice(c * CH, (c + 1) * CH)
                q = 32 * ci
                if m == 0:
                    # trans: dif of latx vs prev kept token.  Col layout
                    # t'*NC + cid*BL + b: prev of t'=0 is (C-1)*NC + col-BL;
                    # cid=0 takes z0.
                    dif = tp.tile([128, CH], BF16, tag=f"difc{tg}{ci}",
                                  name=f"dif{tg}{c}")
                    cst = c * CH
                    if cst >= NC:
                        nc.vector.tensor_sub(dif[:],
                                             latx[:, cst:cst + CH],
                                             latx[:, cst - NC:cst + CH - NC])
                    else:
                        nc.vector.tensor_sub(
                            dif[:, NC:CH], latx[:, NC:CH],
                            latx[:, 0:CH - NC])
                        pbase = (self.C - 1) * NC
                        nc.vector.tensor_sub(
                            dif[:, self.BL:NC], latx[:, self.BL:NC],
                            latx[:, pbase:pbase + NC - self.BL])
                        for bcol in range(self.BL):
                            nc.vector.tensor_sub(
                                dif[:, bcol:bcol + 1],
                                latx[:, bcol:bcol + 1], self.z016[:])
                    d2 = tp.tile([128, CH], BF16, tag=f"sqdc{tg}{ci}",
                                 name=f"d2{tg}{c}")
                    nc.vector.tensor_mul(d2[:], dif[:], dif[:])
                    nc.tensor.matmul(stS[0:q + 1, :], self.sel(q),
                                     d2[:], start=(ci == NS - 1),
                                     stop=(ci == 0),
                                     skip_group_check=(0 < ci < NS - 1))
            yield
        if self.dbg_on and cs_list[0] == 0:
            dt2 = self.dec_pool.tile([1, 2 * CH], F32, name="dbgt2")
            nc.vector.tensor_copy(dt2[0:1, 0:CH], stY[0:1, :])
            nc.vector.tensor_copy(dt2[0:1, CH:2 * CH], stS[0:1, :])
            nc.sync.dma_start(self.d_dbg[:, 5 * CH:7 * CH], dt2[:])
        # lrow segments: [recon' | trans'] per chunk at partition 0
        for ci in range(NS):
            q = 32 * ci
            s = 2 * ci * CH
            nc.vector.tensor_scalar(lrow[0:1, s:s + CH], stY[q:q + 1, :],
                                    1.0 / D, 10.0, ALU.mult, ALU.min)
            nc.vector.tensor_scalar(lrow[0:1, s + CH:s + 2 * CH],
                                    stS[q:q + 1, :],
                                    0.3 / L, 3.0, ALU.mult, ALU.min)
        yield

    # ---- ctx loss + combine (generator, per chunk) ----------------------
    def ctx_q(self, latx, latm, lrow, cs_list):
        nc, CH = self.nc, self.CH
        tp = self.dec_pool
        for ci, c in enumerate(cs_list):
            cs = slice(c * CH, (c + 1) * CH)
            tgc = "A" if ci == 0 else "B"
            st = self.sp.tile([128, CH], F32, tag=f"stY{tgc}",
                              name=f"stcx{c}")
            u3 = tp.tile([128, CH], BF16, tag=f"difc{tgc}0", name=f"cxc{c}")
            nc.vector.tensor_mul(u3[:], latx[:, cs], latm[:, cs])
            nc.tensor.matmul(st[0:65, :], self.sel(64), u3[:],
                             start=True, stop=False)
            u2 = tp.tile([128, CH], BF16, tag=f"r2c{tgc}0", name=f"cxb{c}")
            nc.vector.tensor_mul(u2[:], latm[:, cs], latm[:, cs])
            nc.tensor.matmul(st[0:33, :], self.sel(32), u2[:],
                             start=False, stop=False, skip_group_check=True)
            u = tp.tile([128, CH], BF16, tag=f"rdc{tgc}0", name=f"cxa{c}")
            nc.vector.tensor_mul(u[:], latx[:, cs], latx[:, cs])
            nc.tensor.matmul(st[0:1, :], self.sel(0), u[:],
                             start=False, stop=True, skip_group_check=True)
            if self.dbg_on and c == 0:
                dt_ = self.dec_pool.tile([1, 3 * CH], F32, name="dbgt")
                nc.vector.tensor_copy(dt_[0:1, 0:CH], st[0:1, :])
                nc.vector.tensor_copy(dt_[0:1, CH:2 * CH], st[32:33, :])
                nc.vector.tensor_copy(dt_[0:1, 2 * CH:3 * CH], st[64:65, :])
                nc.sync.dma_start(self.d_dbg[:, 0:3 * CH], dt_[:])
                nc.sync.dma_start(self.d_dbg[:, 3 * CH:5 * CH],
                                  lrow[0:1, 0:2 * CH])
            # rows at partition 0: rx, rm, cos, combine
            rx = self.row_pool.tile([1, CH], F32, tag=f"rt{tgc}c0",
                                    name=f"cxrx{c}")
            rm = self.row_pool.tile([1, CH], F32, tag=f"rt{tgc}c1",
                                    name=f"cxrm{c}")
            nc.scalar.activation(rx[:], st[0:1, :], AF.Sqrt)
            nc.scalar.activation(rm[:], st[32:33, :], AF.Sqrt)
            nc.vector.tensor_scalar_max(rx[:], rx[:], NORM_EPS)
            nc.vector.tensor_scalar_max(rm[:], rm[:], NORM_EPS)
            nc.vector.reciprocal(rx[:], rx[:])
            nc.vector.reciprocal(rm[:], rm[:])
            cosr = self.row_pool.tile([1, CH], F32, tag="cosr",
                                      name=f"cosr{c}")
            nc.vector.tensor_mul(cosr[:], st[64:65, :], rx[:])
            nc.vector.tensor_mul(cosr[:], cosr[:], rm[:])
            # 0.3*clip(1-cos, 0, 10) then + recon' + trans'
            nc.vector.tensor_scalar(cosr[:], cosr[:], -0.3, 0.3,
                                    ALU.mult, ALU.add)
            nc.vector.tensor_scalar(cosr[:], cosr[:], 0.0, 3.0,
                                    ALU.max, ALU.min)
            s = 2 * ci * CH
            nc.vector.tensor_add(cosr[:], cosr[:], lrow[0:1, s:s + CH])
            nc.vector.tensor_add(cosr[:], cosr[:],
                                 lrow[0:1, s + CH:s + 2 * CH])
            nc.sync.dma_start(self.d_out[:, cs], cosr[:])
            yield

    # ---- main build -----------------------------------------------------
    def build(self):
        nc = self.nc
        CH, NCH, NC, T, W, C = (self.CH, self.NCH, self.NC, self.T,
                                self.W, self.C)
        KEPT, BL = self.KEPT, self.BL
        self.declare()
        with tile.TileContext(nc) as tc:
            with (
                tc.tile_pool(name="const", bufs=1) as const_pool,
                tc.tile_pool(name="wenc", bufs=1) as wenc_pool,
                tc.tile_pool(name="big", bufs=1) as big_pool,
                tc.tile_pool(name="tmp", bufs=1) as tmp_pool,
                tc.tile_pool(name="rows", bufs=1) as row_pool,
            ):
                self.tmp_pool, self.row_pool = tmp_pool, row_pool

                # ones at column 96: slicing [96-q:97] puts the ones at
                # column q of the slice -> column sums land on PSUM row q
                self.selq = const_pool.tile([128, 97], BF16, name="selq")
                nc.vector.memset(self.selq[:], 0.0)
                nc.vector.memset(self.selq[:, 96:97], 1.0)
                zcol = const_pool.tile([128, BL], BF16, name="zcol")
                nc.vector.memset(zcol[:], 0.0)
                voff = self.blob_off["vecs"][0]
                vecs16 = const_pool.tile([128, self._vec_cols], BF16,
                                         name="vecs16")
                nc.sync.dma_start(
                    vecs16[:], self.d_blob[:, voff:voff + self._vec_cols])
                self.vecs_sb = const_pool.tile([128, self._vec_cols], F32)
                nc.vector.tensor_copy(self.vecs_sb[:], vecs16[:])
                self.z016 = const_pool.tile([128, 1], BF16, name="z016")
                nc.vector.tensor_copy(self.z016[:], self.vcol("z0"))

                self.w_sb = {}
                for k in ("W1", "W2", "W3", "W4", "W5"):
                    self.w_sb[k] = self.load_weight_tiles(wenc_pool, k)

                latx = big_pool.tile([128, KEPT], BF16, tag="latx",
                                     name="latx")
                latm = big_pool.tile([128, KEPT], BF16, tag="latm",
                                     name="latm")
                memw = big_pool.tile([128, 6 * KEPT], BF16, tag="memw",
                                     name="memw")

                # xtk: kept tokens in chunk-step order
                xtk_cm = tc.tile_pool(name="xtkp", bufs=1)
                xtk_pool = xtk_cm.__enter__()
                xtk = [xtk_pool.tile([128, KEPT], BF16, tag=f"xtk{k}",
                                     name=f"xtk{k}") for k in range(6)]
                off, _, M = self.blob_off["xtk"]
                for k in range(6):
                    nc.sync.dma_start(
                        xtk[k][:],
                        self.d_blob[:, off + k * M:off + (k + 1) * M])

                # ==== phase 0: ptk = Wm^T xtk + bm (kept cols only)
                pt_cm = tc.tile_pool(name="ptp", bufs=1)
                pt_pool = pt_cm.__enter__()
                ptw = pt_pool.tile([128, 6 * KEPT], BF16, tag="ptw",
                                   name="ptw")
                ptk4 = ptw[:].rearrange("p (m t n) -> p m t n", m=6, t=C)
                wm_cm = tc.tile_pool(name="wmp", bufs=1)
                wm_pool = wm_cm.__enter__()
                wm = self.load_weight_tiles(wm_pool, "Wm")
                with tc.tile_pool(name="ps0", bufs=1, space="PSUM") as pp0:
                    for m in range(6):
                        for base in range(0, KEPT, CH):
                            n = min(CH, KEPT - base)
                            ps = pp0.tile([128, CH], F32, tag="p0",
                                          name=f"p0m{m}b{base}", bufs=2)
                            for k in range(6):
                                nc.tensor.matmul(
                                    ps[:, 0:n],
                                    wm[k][:, m * 128:(m + 1) * 128],
                                    xtk[k][:, base:base + n],
                                    start=(k == 0), stop=(k == 5))
                            pb = m * KEPT + base
                            if m % 2 == 0:
                                nc.scalar.activation(
                                    ptw[:, pb:pb + n], ps[:, 0:n],
                                    AF.Identity, bias=self.vcol("bm", m))
                            else:
                                nc.vector.tensor_scalar_add(
                                    ptw[:, pb:pb + n], ps[:, 0:n],
                                    self.vcol("bm", m))

                wm_cm.__exit__(None, None, None)

                # ==== phase 1: recurrence (standalone)
                um_cm = tc.tile_pool(name="ump", bufs=1)
                um_pool = um_cm.__enter__()
                um = self.load_weight_tiles(um_pool, "Um")
                id_off = self.blob_off["id"][0]
                id_sb = um_pool.tile([128, 128], BF16, name="id_sb")
                nc.sync.dma_start(id_sb[:],
                                  self.d_blob[:, id_off:id_off + 128])
                stp_cm = tc.tile_pool(name="stp", bufs=1)
                stp_pool = stp_cm.__enter__()
                rps_cm = tc.tile_pool(name="recps", bufs=1, space="PSUM")
                self.rps = rps_cm.__enter__()
                stpv = [stp_pool.tile([128, 6 * NC], BF16, tag=f"stp{i}",
                                      name=f"stp{i}")[:].rearrange(
                            "p (m n) -> p m n", m=6)
                        for i in range(2)]
                memc4 = memw[:].rearrange("p (m t n) -> p m t n", m=6, t=C)
                for _ in self.recurrence_q(ptk4, ptw[:], memc4, um,
                                           id_sb, zcol, stpv):
                    pass
                rps_cm.__exit__(None, None, None)
                stp_cm.__exit__(None, None, None)
                um_cm.__exit__(None, None, None)
                pt_cm.__exit__(None, None, None)

                # long-lived MLP PSUM pools (phases 2-5)
                mlp_ps = tc.tile_pool(name="mps", bufs=1, space="PSUM")
                self.pp = mlp_ps.__enter__()
                mlp_sp = tc.tile_pool(name="msp", bufs=1, space="PSUM")
                self.sp = mlp_sp.__enter__()

                # ==== phase 2: encode(x) || encode(mem), half-width streams
                allc = list(range(NCH))
                xt_chs = {c: [xtk[k][:, c * CH:(c + 1) * CH]
                              for k in range(6)] for c in allc}
                lat_ov = {c: latx[:, c * CH:(c + 1) * CH] for c in allc}
                latm_ov = {c: latm[:, c * CH:(c + 1) * CH] for c in allc}
                memv = memw[:].rearrange("p (m tn) -> p m tn", m=6)
                mem_chs = {c: [memv[:, k, c * CH:(c + 1) * CH]
                               for k in range(6)] for c in allc}
                h0 = max(1, NCH // 2)
                import os as _os
                if _os.environ.get("SKIP_ENC") == "1":
                    nc.vector.memset(latx[:], 0.1)
                    nc.vector.memset(latm[:], 0.1)
                else:
                    g_x = _chain(
                        self.encode_q("x", xt_chs, lat_ov, BF16, allc[:h0]),
                        self.encode_q("x", xt_chs, lat_ov, BF16, allc[h0:])
                        if NCH > 1 else None)
                    g_m = _chain(
                        self.encode_q("m", mem_chs, latm_ov, BF16,
                                      allc[:h0]),
                        self.encode_q("m", mem_chs, latm_ov, BF16,
                                      allc[h0:])
                        if NCH > 1 else None)
                    _interleave([g_x, g_m], [1, 1])

                # ==== phase 3: decode + recon/trans (2 half passes)
                wdec_cm = tc.tile_pool(name="wdec", bufs=1)
                wdec_pool = wdec_cm.__enter__()
                self.dec_pool = wdec_pool
                halves = [allc[:h0], allc[h0:]] if NCH > 1 else [allc]
                lrows = [wdec_pool.tile([1, 2 * len(csl) * CH], F32,
                                        tag=f"lrow{h}", name=f"lrow{h}")
                         for h, csl in enumerate(halves)]
                wd = [self.load_weight_tiles(wdec_pool, k)
                      for k in ("Wd1", "Wd2", "Wd3")]
                if _os.environ.get("SKIP_DEC") == "1":
                    for lr in lrows:
                        nc.vector.memset(lr[:], 0.1)
                else:
                    decs = [self.decode_q("d" if h == 0 else "d2", latx,
                                          xtk, wd, lrows[h], csl)
                            for h, csl in enumerate(halves)]
                    _interleave(decs, [1] * len(decs))

                # ==== phase 5: ctx + combine + output
                for h, csl in enumerate(halves):
                    for _ in self.ctx_q(latx, latm, lrows[h], csl):
                        pass

                wdec_cm.__exit__(None, None, None)
                mlp_sp.__exit__(None, None, None)
                mlp_ps.__exit__(None, None, None)
                xtk_cm.__exit__(None, None, None)
        nc.compile()
        return nc


def _chain(*gens):
    for g in gens:
        if g is not None:
            yield from g


def _interleave(gens, weights):
    gens = list(gens)
    weights = list(weights)
    while gens:
        for i in range(len(gens) - 1, -1, -1):
            try:
                for _ in range(weights[i]):
                    next(gens[i])
            except StopIteration:
                del gens[i]
                del weights[i]


# ---------------------------------------------------------------- runner

_CACHE = {}


def _get_built(S, BL):
    key = (S, BL)
    if key not in _CACHE:
        kb = _KB(S, BL)
        kb.build()
        _CACHE[key] = kb
    return _CACHE[key]


def _host_inputs(kb, inputs):
    S, BL, C, W, T, NC = kb.S, kb.BL, kb.C, kb.W, kb.T, kb.NC
    w = {k: np.asarray(v, np.float32) for k, v in inputs.items()}
    Wvo = w["Wv"] @ w["Wo"]
    bvo = w["bv"] @ w["Wo"] + w["bo"]
    wd = dict(w)
    # fold the (linear) self-attn projection into W2: a@W2 = h@(Wvo@W2)
    wd["W2"] = Wvo @ w["W2"]
    wd["b2"] = bvo @ w["W2"] + w["b2"]
    z0 = _encode_np(np.zeros((1, D), np.float32), wd)[0]

    vecs = _pack_cols(w["b1"], w["g1"], w["be1"],
                      wd["b2"], w["g2"], w["be2"],
                      w["b3"], w["g3"], w["be3"],
                      w["b4"], w["g4"], w["be4"],
                      w["b5"], w["g5"], w["be5"],
                      w["bd1"], w["gd1"], w["bed1"],
                      w["bd2"], w["gd2"], w["bed2"],
                      w["bd3"], w["bm"], z0,
                      np.full(128, LN_EPS, np.float32))

    def b16(x):
        return np.ascontiguousarray(x.astype(ml_dtypes.bfloat16))

    wd["id"] = np.eye(128, dtype=np.float32)
    wd["vecs"] = vecs
    blob_off, nblob = kb.blob_layout()
    wblob = np.zeros((128, nblob), ml_dtypes.bfloat16)
    for name, (off, ntiles, M) in blob_off.items():
        if name == "xtk":
            continue
        wsrc = np.asarray(wd[name], np.float32)
        for k in range(ntiles):
            wblob[:, off + k * M:off + (k + 1) * M] = b16(
                wsrc[k * 128:(k + 1) * 128, :])

    # kept tokens in chunk-step order: col(t', cid, b) = t'*NC + cid*BL + b
    seqs = np.asarray(inputs["sequences"], np.float32)
    ncid = S // C
    in_maps = []
    xtk_off = blob_off["xtk"]
    for core in range(NCORES):
        xs = seqs[core * BL:(core + 1) * BL, :S, :]       # [BL,S,D]
        g = xs.reshape(BL, ncid, C, D)                    # [BL,cid,t',D]
        g = np.transpose(g, (2, 1, 0, 3))                 # [t',cid,BL,D]
        gt16 = b16(g.reshape(kb.KEPT, D).T)               # [D, KEPT]
        blob = wblob.copy()
        off, _, M = xtk_off
        for k in range(6):
            blob[:, off + k * M:off + (k + 1) * M] = \
                gt16[k * 128:(k + 1) * 128, :]
        in_maps.append(dict(blob16=blob))
    return in_maps


def _l2_term(inputs):
    names = ["W1", "b1", "g1", "be1", "Wv", "bv", "Wo", "bo", "W2", "b2",
             "g2", "be2", "W3", "b3", "g3", "be3", "W4", "b4", "g4", "be4",
             "W5", "b5", "g5", "be5", "Wd1", "bd1", "gd1", "bed1", "Wd2",
             "bd2", "gd2", "bed2", "Wd3", "bd3", "Wm", "Um", "bm"]
    l2 = sum(np.linalg.norm(np.asarray(inputs[n], np.float64))
             for n in names)
    return float(np.clip(l2, 0.0, 10.0))


def _combine(kb, res, inputs):
    tok = np.concatenate([res.results[c]["tok_loss"].reshape(-1)
                          for c in range(NCORES)])
    l2 = _l2_term(inputs)
    per_tok = np.clip(tok.astype(np.float64) + 1e-4 * l2, 0.0, 100.0)
    nb = kb.BL * NCORES
    return np.float32(per_tok.sum() / nb)


def kernel(**inputs):
    seqs = np.asarray(inputs["sequences"])
    S = seqs.shape[1]
    BL = seqs.shape[0] // NCORES
    kb = _get_built(S, BL)
    in_maps = _host_inputs(kb, inputs)
    res = run_bass_kernel_spmd(kb.nc, in_maps, list(range(NCORES)))
    return _combine(kb, res, inputs)


# revision 11
# speedup vs baseline: 1.0146x; 1.0077x over previous
"""Trainium2 Bass kernel for nn_EmotionalEmbeddingSpace (v2).

Sharding: data-parallel over batch B=16 across 8 cores (BL=2 sequences/core).
Layout: features on partitions, tokens on the free dim, in *chunk-step*
column order: col(t', cid, b) = t'*NC + cid*BL + b.

The tanh memory recurrence contracts at ~0.45/step, so each sequence is cut
into S/C chunks of C=16 positions, each warmed up from state=0 over W=16
extra steps (approximation error ~3e-6, far below bf16 noise).  All
BL*S/C = 128 chunks advance together: serial depth drops 1024 -> 32 and each
step's matmuls are 128 columns wide.  With W == C, the warmup-step pt values
are exactly the kept pt columns shifted by BL (chunk cid warms up over chunk
cid-1's positions), so no duplicate storage is needed; chunk 0 warms up on
injected zeros, which reproduces the reference's mem_{-1} = 0 exactly.

LN per layer: y evac on ScalarE, y^2/apply on DVE, column stats via
ones-column matmuls stacked into one PSUM tile (chunk c -> partition rows
c / 32+c), row math on [NCH, *] lanes at once, mean/rstd broadcast on
GpSimd, relu+bias via tensor_scalar.  encode(x) and encode(mem) run as two
interleaved half-width streams, as do the two decode halves, so serial
row-math bubbles on one chain are filled by the other.
"""

import sys

sys.path.insert(0, "/opt/trn_rl_repo")

import numpy as np
import ml_dtypes

import concourse.bass as bass
import concourse.bacc as bacc
import concourse.mybir as mybir
import concourse.tile as tile
from concourse.bass_utils import run_bass_kernel_spmd

F32 = mybir.dt.float32
BF16 = mybir.dt.bfloat16
AF = mybir.ActivationFunctionType
ALU = mybir.AluOpType

B, S_FULL, D, H, L = 16, 1024, 768, 512, 128
NCORES = 8
LN_EPS = 1e-5
NORM_EPS = 1e-8
CREC = 16   # chunk length
WREC = 16   # warmup length (must equal CREC for the shift trick)


# ---------------------------------------------------------------- host prep

def _pack_cols(*vecs):
    cols = []
    for v in vecs:
        v = np.asarray(v, np.float32).reshape(-1, 128)
        cols.append(v.T)
    return np.ascontiguousarray(np.concatenate(cols, axis=1))


def _ln_np(x, g, b, eps=LN_EPS):
    m = x.mean(-1, keepdims=True)
    v = ((x - m) ** 2).mean(-1, keepdims=True)
    return (x - m) / np.sqrt(v + eps) * g + b


def _encode_np(t, w):
    """w["W2"]/w["b2"] are the Wvo-folded effective weights."""
    h = np.maximum(_ln_np(t @ w["W1"] + w["b1"], w["g1"], w["be1"]), 0)
    g = np.maximum(_ln_np(h @ w["W2"] + w["b2"], w["g2"], w["be2"]), 0)
    zl = _ln_np(g @ w["W3"] + w["b3"], w["g3"], w["be3"])
    e = np.maximum(_ln_np(zl @ w["W4"] + w["b4"], w["g4"], w["be4"]), 0)
    return _ln_np(e @ w["W5"] + w["b5"], w["g5"], w["be5"])


# ---------------------------------------------------------------- builder

class _KB:
    WSHAPES = dict(W1=(D, H), W2=(H, H), W3=(H, L), W4=(L, H),
                   W5=(H, L), Wd1=(L, H), Wd2=(H, H), Wd3=(H, D),
                   Wm=(D, D), Um=(D, D))

    def __init__(self, S=S_FULL, BL=B // NCORES):
        self.S, self.BL = S, BL
        self.C, self.W = CREC, WREC
        assert self.C == self.W
        self.T = self.C + self.W
        self.NC = BL * S // self.C          # chunk columns per step
        self.KEPT = BL * S                  # kept token columns
        self.CH = min(512, self.KEPT)
        self.NCH = self.KEPT // self.CH
        assert self.NCH <= 16
        self.nc = bacc.Bacc("TRN2", target_bir_lowering=False, debug=False,
                            num_devices=NCORES)
        self.vec_map = {}
        self._vec_cols = 0
        self.layer_ctr = 0

    def _reg_vec(self, name, ntiles):
        self.vec_map[name] = (self._vec_cols, ntiles)
        self._vec_cols += ntiles

    def blob_layout(self):
        entries = [("xtk", 6, self.KEPT)]
        for k, (K, M) in self.WSHAPES.items():
            entries.append((k, K // 128, M))
        entries.append(("id", 1, 128))
        entries.append(("vecs", 1, self._vec_cols))
        off = {}
        pos = 0
        for name, ntiles, M in entries:
            off[name] = (pos, ntiles, M)
            pos += ntiles * M
        return off, pos

    def declare(self):
        nc = self.nc
        for nm, n in [("b1", 4), ("g1", 4), ("be1", 4),
                      ("b2", 4), ("g2", 4), ("be2", 4),
                      ("b3", 1), ("g3", 1), ("be3", 1),
                      ("b4", 4), ("g4", 4), ("be4", 4),
                      ("b5", 1), ("g5", 1), ("be5", 1),
                      ("bd1", 4), ("gd1", 4), ("bed1", 4),
                      ("bd2", 4), ("gd2", 4), ("bed2", 4),
                      ("bd3", 6), ("bm", 6), ("z0", 1), ("lneps", 1)]:
            self._reg_vec(nm, n)
        self.blob_off, nblob = self.blob_layout()
        self.d_blob = nc.dram_tensor("blob16", [128, nblob], BF16,
                                     kind="ExternalInput")
        self.d_out = nc.dram_tensor("tok_loss", [1, self.KEPT], F32,
                                    kind="ExternalOutput")
        import os as _os
        self.dbg_on = _os.environ.get("DBG_DUMP") == "1"
        if self.dbg_on:
            self.d_dbg = nc.dram_tensor("dbg", [1, 8 * self.CH], F32,
                                        kind="ExternalOutput")

    def vcol(self, name, t=0, rows=128):
        s, n = self.vec_map[name]
        assert t < n
        return self.vecs_sb[0:rows, s + t:s + t + 1]

    # ---- helpers --------------------------------------------------------
    def sel(self, q):
        """Stationary that sums columns onto PSUM row q (out rows 0..q)."""
        return self.selq[:, 96 - q:97]

    def load_weight_tiles(self, pool, wname):
        nc = self.nc
        off, ntiles, M = self.blob_off[wname]
        tiles = []
        for k in range(ntiles):
            t = pool.tile([128, M], BF16, tag=f"w_{wname}_{k}",
                          name=f"w_{wname}_{k}")
            nc.sync.dma_start(
                t[:], self.d_blob[:, off + k * M:off + (k + 1) * M])
            tiles.append(t)
        return tiles

    # ---- balanced LN layer (generator yielding per issue quantum) -------
    def layer_q(self, sid, chs_in, w_tiles, M_out, *, bias, ln=None,
                relu=False, out_override=None, out_dtype=BF16, out_slot=0,
                cs_list=None):
        nc, CH = self.nc, self.CH
        if cs_list is None:
            cs_list = sorted(chs_in.keys())
        NC_ST = len(cs_list)
        tg = {"x": "A", "m": "B", "d": "A", "d2": "B"}[sid]
        n_k = len(chs_in[cs_list[0]])
        n_m = M_out // 128
        tp = self.tmp_pool
        outs = {}
        for ci, c in enumerate(cs_list):
            if out_override is not None:
                outs[c] = [out_override[c]]
            else:
                outs[c] = [tp.tile([128, CH], out_dtype,
                                   tag=f"o{tg}{out_slot}m{m}c{ci}",
                                   name=f"o{tg}{out_slot}m{m}c{c}")[:]
                           for m in range(n_m)]
        if ln is None:
            for m in range(n_m):
                for c in cs_list:
                    ps = self.pp.tile([128, CH], F32, tag=f"ps{tg}",
                                      name=f"ps{tg}m{m}c{c}", bufs=2)
                    for k in range(n_k):
                        nc.tensor.matmul(ps[:],
                                         w_tiles[k][:, m * 128:(m + 1) * 128],
                                         chs_in[c][k], start=(k == 0),
                                         stop=(k == n_k - 1))
                    if (m + c) % 2 == 0:
                        nc.scalar.activation(outs[c][m], ps[:],
                                             AF.Relu if relu else AF.Identity,
                                             bias=self.vcol(bias, m))
                    elif relu:
                        nc.vector.tensor_scalar(
                            outs[c][m], ps[:], self.vcol(bias, m), 0.0,
                            ALU.add, ALU.max)
                    else:
                        nc.vector.tensor_scalar_add(
                            outs[c][m], ps[:], self.vcol(bias, m))
                yield
            return outs
        g_nm, be_nm = ln
        self.layer_ctr += 1
        lid = self.layer_ctr
        stY = self.sp.tile([128, CH], F32, tag=f"stY{tg}",
                           name=f"stY{tg}{lid}")
        stS = self.sp.tile([128, CH], F32, tag=f"stS{tg}",
                           name=f"stS{tg}{lid}")
        W2 = NC_ST * CH
        # fused per-m tiles spanning all chunks of this half
        ym = [tp.tile([128, W2], BF16, tag=f"y{tg}m{m}", name=f"y{tg}{lid}m{m}")
              for m in range(n_m)]
        n_mm = n_m * NC_ST
        mm_i = 0
        ci_order = list(enumerate(cs_list))[::-1]  # widest stats MM first
        for m in range(n_m):
            sq = tp.tile([128, W2], BF16, tag=f"sq{tg}", name=f"sq{tg}{lid}m{m}")
            for ci, c in ci_order:
                ps = self.pp.tile([128, CH], F32, tag=f"ps{tg}",
                                  name=f"ps{tg}m{m}c{c}", bufs=2)
                for k in range(n_k):
                    nc.tensor.matmul(ps[:],
                                     w_tiles[k][:, m * 128:(m + 1) * 128],
                                     chs_in[c][k], start=(k == 0),
                                     stop=(k == n_k - 1))
                ysl = ym[m][:, ci * CH:(ci + 1) * CH]
                sqs = sq[:, ci * CH:(ci + 1) * CH]
                nc.scalar.activation(ysl, ps[:], AF.Identity,
                                     bias=self.vcol(bias, m))
                if m % 2 == 0:
                    nc.scalar.activation(sqs, ps[:], AF.Square,
                                         bias=self.vcol(bias, m))
                else:
                    nc.vector.tensor_mul(sqs, ysl, ysl)
                q = 32 * ci
                nc.tensor.matmul(stY[0:q + 1, :], self.sel(q), ysl,
                                 start=(mm_i == 0), stop=(mm_i == n_mm - 1),
                                 skip_group_check=(0 < mm_i < n_mm - 1))
                nc.tensor.matmul(stS[0:q + 1, :], self.sel(q),
                                 sq[:, ci * CH:(ci + 1) * CH],
                                 start=(mm_i == 0), stop=(mm_i == n_mm - 1),
                                 skip_group_check=(0 < mm_i < n_mm - 1))
                mm_i += 1
            yield
        # per-chunk row math at partition 0 into one combined row:
        # [mean c0 | mean c1 | rstd c0 | rstd c1]
        inv_f = 1.0 / M_out
        r16 = self.row_pool.tile([1, 2 * W2], BF16, tag=f"r16{tg}",
                                 name=f"r16{tg}{lid}")
        for ci, c in enumerate(cs_list):
            q = 32 * ci
            rt = self.row_pool.tile([1, CH], F32, tag=f"rt{tg}c{ci}",
                                    name=f"rt{tg}{lid}c{c}")
            mseg = r16[0:1, ci * CH:(ci + 1) * CH]
            rseg = r16[0:1, W2 + ci * CH:W2 + (ci + 1) * CH]
            nc.scalar.activation(mseg, stY[q:q + 1, :], AF.Copy,
                                 scale=inv_f)
            nc.vector.scalar_tensor_tensor(rt[:], mseg, -1.0, mseg,
                                           ALU.mult, ALU.mult)
            nc.vector.scalar_tensor_tensor(rt[:], stS[q:q + 1, :], inv_f,
                                           rt[:], ALU.mult, ALU.add)
            nc.scalar.activation(rt[:], rt[:], AF.Sqrt,
                                 bias=self.vcol("lneps", rows=1))
            with nc.allow_low_precision(reason="bf16 rstd row, 0.4% rel"):
                nc.vector.reciprocal(rseg, rt[:])
        bc = tp.tile([128, 2 * W2], BF16, tag=f"bc{tg}",
                     name=f"bc{tg}{lid}")
        nc.gpsimd.partition_broadcast(bc[:], r16[:])
        yield
        # fused apply over all chunks: out = act((y - meanb)*g*rstd_b + be)
        meanb = bc[:, 0:W2]
        rstd_b = bc[:, W2:2 * W2]
        for m in range(n_m):
            u = tp.tile([128, W2], BF16, tag=f"u{tg}", name=f"u{tg}{lid}m{m}")
            nc.vector.tensor_sub(u[:], ym[m][:], meanb)
            nc.vector.scalar_tensor_tensor(u[:], u[:], self.vcol(g_nm, m),
                                           rstd_b, ALU.mult, ALU.mult)
            for ci, c in enumerate(cs_list):
                usl = u[:, ci * CH:(ci + 1) * CH]
                if relu:
                    nc.vector.tensor_scalar(
                        outs[c][m], usl, self.vcol(be_nm, m), 0.0,
                        ALU.add, ALU.max)
                else:
                    nc.vector.tensor_scalar_add(
                        outs[c][m], usl, self.vcol(be_nm, m))
            yield
        return outs

    def encode_q(self, sid, chs_in, out_override, out_dtype, cs_list):
        h = yield from self.layer_q(sid, chs_in, self.w_sb["W1"], H,
                                    bias="b1", ln=("g1", "be1"), relu=True,
                                    out_slot=0, cs_list=cs_list)
        g = yield from self.layer_q(sid, h, self.w_sb["W2"], H, bias="b2",
                                    ln=("g2", "be2"), relu=True, out_slot=1,
                                    cs_list=cs_list)
        zl = yield from self.layer_q(sid, g, self.w_sb["W3"], L, bias="b3",
                                     ln=("g3", "be3"), out_slot=0,
                                     cs_list=cs_list)
        e = yield from self.layer_q(sid, zl, self.w_sb["W4"], H, bias="b4",
                                    ln=("g4", "be4"), relu=True, out_slot=1,
                                    cs_list=cs_list)
        yield from self.layer_q(sid, e, self.w_sb["W5"], L, bias="b5",
                                ln=("g5", "be5"), out_override=out_override,
                                out_dtype=out_dtype, cs_list=cs_list)

    # ---- recurrence (generator yielding per step) -----------------------
    def recurrence_q(self, ptk4, ptkf, memc4, um, id_sb, zcol, stpv):
        """ptk4: [p, m, C, NC] kept pt; ptkf: flat [p, 6*KEPT] view;
        memc4: [p, m, C, NC] mem output; stpv: two [p, m, NC] scratch."""
        nc = self.nc
        NC, T, W, BL, KEPT = self.NC, self.T, self.W, self.BL, self.KEPT
        import os as _os
        if _os.environ.get("SKIP_REC") == "1":
            nc.vector.memset(memc4[:, :, :, :], 0.1)
            return
        G = 3 * NC

        def dst(t, g):
            if t < W:
                return stpv[t % 2][:, 3 * g:3 * g + 3, :]
            return memc4[:, 3 * g:3 * g + 3, t - W, :]

        def src(t, k):
            if t < W:
                return stpv[t % 2][:, k, :]
            return memc4[:, k, t - W, :]

        for t in range(T):
            pss = []
            for g in range(2):
                ps = self.rps.tile([128, G], F32, tag=f"rps{g}",
                                   name=f"rps{g}t{t}", bufs=2,
                                   padded_shape=[128, 512])
                for mi in range(3):
                    m = 3 * g + mi
                    if t < W:
                        # warmup: chunk cid reads chunk cid-1's kept pt
                        # (W == C); chunk 0 gets zeros.
                        nc.tensor.matmul(
                            ps[:, mi * NC:mi * NC + BL], id_sb[:],
                            zcol[:], start=(mi == 0), stop=False,
                            skip_group_check=(mi != 0))
                        last = (t == 0 and mi == 2)
                        base = m * KEPT + t * NC
                        nc.tensor.matmul(
                            ps[:, mi * NC + BL:(mi + 1) * NC], id_sb[:],
                            ptkf[:, base:base + NC - BL],
                            start=False, stop=last,
                            skip_group_check=not last)
                    else:
                        nc.tensor.matmul(
                            ps[:, mi * NC:(mi + 1) * NC], id_sb[:],
                            ptk4[:, m, t - W, :],
                            start=(mi == 0), stop=False,
                            skip_group_check=(mi != 0))
                pss.append(ps)
            for g in range(2):
                ps = pss[g]
                if t > 0:
                    for k in range(6):
                        for mi in range(3):
                            m = 3 * g + mi
                            last = (k == 5 and mi == 2)
                            nc.tensor.matmul(
                                ps[:, mi * NC:(mi + 1) * NC],
                                um[k][:, m * 128:(m + 1) * 128],
                                src(t - 1, k),
                                start=False, stop=last,
                                skip_group_check=not last)
                psv = ps[:].rearrange("p (m n) -> p m n", m=3)
                nc.scalar.activation(dst(t, g), psv[:], AF.Tanh)
            yield

    # ---- decode + recon/trans losses (generator, over a chunk subset) ---
    def decode_q(self, sid, latx, xtk, wd, lrow, cs_list):
        """lrow: [1, 2*NS*CH] partition-0 segments, per chunk:
        [recon' | 0.3*trans'] at cols [2*ci*CH, (2*ci+2)*CH)."""
        nc, CH, NC = self.nc, self.CH, self.NC
        tg = {"d": "A", "d2": "B"}[sid]
        lat16 = {c: [latx[:, c * CH:(c + 1) * CH]] for c in cs_list}
        h1 = yield from self.layer_q(sid, lat16, wd[0], H, bias="bd1",
                                     ln=("gd1", "bed1"), relu=True,
                                     out_slot=0, cs_list=cs_list)
        h2 = yield from self.layer_q(sid, h1, wd[1], H, bias="bd2",
                                     ln=("gd2", "bed2"), relu=True,
                                     out_slot=1, cs_list=cs_list)
        NS = len(cs_list)
        stY = self.sp.tile([128, CH], F32, tag=f"stY{tg}",
                           name=f"strcY{cs_list[0]}")
        stS = self.sp.tile([128, CH], F32, tag=f"stS{tg}",
                           name=f"strcS{cs_list[0]}")
        tp = self.dec_pool
        n_mm = 6 * NS
        mm_i = 0
        W2r = NS * CH
        for m in range(6):
            r = tp.tile([128, W2r], BF16, tag=f"rdc{tg}",
                        name=f"rd{tg}{m}")
            r2 = tp.tile([128, W2r], BF16, tag=f"r2c{tg}",
                         name=f"r2{tg}{m}")
            for ci, c in list(enumerate(cs_list))[::-1]:
                cs = slice(c * CH, (c + 1) * CH)
                ps = self.pp.tile([128, CH], F32, tag=f"ps{tg}",
                                  name=f"psd{m}c{c}", bufs=2)
                for k in range(4):
                    nc.tensor.matmul(ps[:],
                                     wd[2][k][:, m * 128:(m + 1) * 128],
                                     h2[c][k], start=(k == 0), stop=(k == 3))
                nc.vector.scalar_tensor_tensor(
                    r[:, ci * CH:(ci + 1) * CH], ps[:], self.vcol("bd3", m),
                    xtk[m][:, cs], ALU.add, ALU.subtract)
            nc.vector.tensor_mul(r2[:], r[:], r[:])
            for ci, c in list(enumerate(cs_list))[::-1]:
                q = 32 * ci
                nc.tensor.matmul(stY[0:q + 1, :], self.sel(q),
                                 r2[:, ci * CH:(ci + 1) * CH],
                                 start=(mm_i == 0), stop=(mm_i == n_mm - 1),
                                 skip_group_check=(0 < mm_i < n_mm - 1))
                mm_i += 1
            for ci, c in list(enumerate(cs_list))[::-1]:
                cs = slice(c * CH, (c + 1) * CH)
                q = 32 * ci
                if m == 0:
                    # trans: dif of latx vs prev kept token.  Col layout
                    # t'*NC + cid*BL + b: prev of t'=0 is (C-1)*NC + col-BL;
                    # cid=0 takes z0.
                    dif = tp.tile([128, CH], BF16, tag=f"difc{tg}{ci}",
                                  name=f"dif{tg}{c}")
                    cst = c * CH
                    if cst >= NC:
                        nc.vector.tensor_sub(dif[:],
                                             latx[:, cst:cst + CH],
                                             latx[:, cst - NC:cst + CH - NC])
                    else:
                        nc.vector.tensor_sub(
                            dif[:, NC:CH], latx[:, NC:CH],
                            latx[:, 0:CH - NC])
                        pbase = (self.C - 1) * NC
                        nc.vector.tensor_sub(
                            dif[:, self.BL:NC], latx[:, self.BL:NC],
                            latx[:, pbase:pbase + NC - self.BL])
                        for bcol in range(self.BL):
                            nc.vector.tensor_sub(
                                dif[:, bcol:bcol + 1],
                                latx[:, bcol:bcol + 1], self.z016[:])
                    d2 = tp.tile([128, CH], BF16, tag=f"sqdc{tg}{ci}",
                                 name=f"d2{tg}{c}")
                    nc.vector.tensor_mul(d2[:], dif[:], dif[:])
                    nc.tensor.matmul(stS[0:q + 1, :], self.sel(q),
                                     d2[:], start=(ci == NS - 1),
                                     stop=(ci == 0),
                                     skip_group_check=(0 < ci < NS - 1))
            yield
        if self.dbg_on and cs_list[0] == 0:
            dt2 = self.dec_pool.tile([1, 2 * CH], F32, name="dbgt2")
            nc.vector.tensor_copy(dt2[0:1, 0:CH], stY[0:1, :])
            nc.vector.tensor_copy(dt2[0:1, CH:2 * CH], stS[0:1, :])
            nc.sync.dma_start(self.d_dbg[:, 5 * CH:7 * CH], dt2[:])
        # lrow segments: [recon' | trans'] per chunk at partition 0
        for ci in range(NS):
            q = 32 * ci
            s = 2 * ci * CH
            nc.vector.tensor_scalar(lrow[0:1, s:s + CH], stY[q:q + 1, :],
                                    1.0 / D, 10.0, ALU.mult, ALU.min)
            nc.vector.tensor_scalar(lrow[0:1, s + CH:s + 2 * CH],
                                    stS[q:q + 1, :],
                                    0.3 / L, 3.0, ALU.mult, ALU.min)
        yield

    # ---- ctx loss + combine (generator, per chunk) ----------------------
    def ctx_q(self, latx, latm, lrow, cs_list):
        nc, CH = self.nc, self.CH
        tp = self.dec_pool
        for ci, c in enumerate(cs_list):
            cs = slice(c * CH, (c + 1) * CH)
            tgc = "A" if ci == 0 else "B"
            st = self.sp.tile([128, CH], F32, tag=f"stY{tgc}",
                              name=f"stcx{c}")
            u3 = tp.tile([128, CH], BF16, tag=f"difc{tgc}0", name=f"cxc{c}")
            nc.vector.tensor_mul(u3[:], latx[:, cs], latm[:, cs])
            nc.tensor.matmul(st[0:65, :], self.sel(64), u3[:],
                             start=True, stop=False)
            u2 = tp.tile([128, CH], BF16, tag=f"r2c{tgc}0", name=f"cxb{c}")
            nc.vector.tensor_mul(u2[:], latm[:, cs], latm[:, cs])
            nc.tensor.matmul(st[0:33, :], self.sel(32), u2[:],
                             start=False, stop=False, skip_group_check=True)
            u = tp.tile([128, CH], BF16, tag=f"rdc{tgc}0", name=f"cxa{c}")
            nc.vector.tensor_mul(u[:], latx[:, cs], latx[:, cs])
            nc.tensor.matmul(st[0:1, :], self.sel(0), u[:],
                             start=False, stop=True, skip_group_check=True)
            if self.dbg_on and c == 0:
                dt_ = self.dec_pool.tile([1, 3 * CH], F32, name="dbgt")
                nc.vector.tensor_copy(dt_[0:1, 0:CH], st[0:1, :])
                nc.vector.tensor_copy(dt_[0:1, CH:2 * CH], st[32:33, :])
                nc.vector.tensor_copy(dt_[0:1, 2 * CH:3 * CH], st[64:65, :])
                nc.sync.dma_start(self.d_dbg[:, 0:3 * CH], dt_[:])
                nc.sync.dma_start(self.d_dbg[:, 3 * CH:5 * CH],
                                  lrow[0:1, 0:2 * CH])
            # rows at partition 0: rx, rm, cos, combine
            rx = self.row_pool.tile([1, CH], F32, tag=f"rt{tgc}c0",
                                    name=f"cxrx{c}")
            rm = self.row_pool.tile([1, CH], F32, tag=f"rt{tgc}c1",
                                    name=f"cxrm{c}")
            nc.scalar.activation(rx[:], st[0:1, :], AF.Sqrt)
            nc.scalar.activation(rm[:], st[32:33, :], AF.Sqrt)
            nc.vector.tensor_scalar_max(rx[:], rx[:], NORM_EPS)
            nc.vector.tensor_scalar_max(rm[:], rm[:], NORM_EPS)
            nc.vector.reciprocal(rx[:], rx[:])
            nc.vector.reciprocal(rm[:], rm[:])
            cosr = self.row_pool.tile([1, CH], F32, tag="cosr",
                                      name=f"cosr{c}")
            nc.vector.tensor_mul(cosr[:], st[64:65, :], rx[:])
            nc.vector.tensor_mul(cosr[:], cosr[:], rm[:])
            # 0.3*clip(1-cos, 0, 10) then + recon' + trans'
            nc.vector.tensor_scalar(cosr[:], cosr[:], -0.3, 0.3,
                                    ALU.mult, ALU.add)
            nc.vector.tensor_scalar(cosr[:], cosr[:], 0.0, 3.0,
                                    ALU.max, ALU.min)
            s = 2 * ci * CH
            nc.vector.tensor_add(cosr[:], cosr[:], lrow[0:1, s:s + CH])
            nc.vector.tensor_add(cosr[:], cosr[:],
                                 lrow[0:1, s + CH:s + 2 * CH])
            nc.sync.dma_start(self.d_out[:, cs], cosr[:])
            yield

    # ---- main build -----------------------------------------------------
    def build(self):
        nc = self.nc
        CH, NCH, NC, T, W, C = (self.CH, self.NCH, self.NC, self.T,
                                self.W, self.C)
        KEPT, BL = self.KEPT, self.BL
        self.declare()
        with tile.TileContext(nc) as tc:
            with (
                tc.tile_pool(name="const", bufs=1) as const_pool,
                tc.tile_pool(name="wenc", bufs=1) as wenc_pool,
                tc.tile_pool(name="big", bufs=1) as big_pool,
                tc.tile_pool(name="tmp", bufs=1) as tmp_pool,
                tc.tile_pool(name="rows", bufs=1) as row_pool,
            ):
                self.tmp_pool, self.row_pool = tmp_pool, row_pool

                # ones at column 96: slicing [96-q:97] puts the ones at
                # column q of the slice -> column sums land on PSUM row q
                self.selq = const_pool.tile([128, 97], BF16, name="selq")
                nc.vector.memset(self.selq[:], 0.0)
                nc.vector.memset(self.selq[:, 96:97], 1.0)
                zcol = const_pool.tile([128, BL], BF16, name="zcol")
                nc.vector.memset(zcol[:], 0.0)
                voff = self.blob_off["vecs"][0]
                vecs16 = const_pool.tile([128, self._vec_cols], BF16,
                                         name="vecs16")
                nc.sync.dma_start(
                    vecs16[:], self.d_blob[:, voff:voff + self._vec_cols])
                self.vecs_sb = const_pool.tile([128, self._vec_cols], F32)
                nc.vector.tensor_copy(self.vecs_sb[:], vecs16[:])
                self.z016 = const_pool.tile([128, 1], BF16, name="z016")
                nc.vector.tensor_copy(self.z016[:], self.vcol("z0"))

                self.w_sb = {}
                for k in ("W1", "W2", "W3", "W4", "W5"):
                    self.w_sb[k] = self.load_weight_tiles(wenc_pool, k)

                latx = big_pool.tile([128, KEPT], BF16, tag="latx",
                                     name="latx")
                latm = big_pool.tile([128, KEPT], BF16, tag="latm",
                                     name="latm")
                memw = big_pool.tile([128, 6 * KEPT], BF16, tag="memw",
                                     name="memw")

                # xtk: kept tokens in chunk-step order
                xtk_cm = tc.tile_pool(name="xtkp", bufs=1)
                xtk_pool = xtk_cm.__enter__()
                xtk = [xtk_pool.tile([128, KEPT], BF16, tag=f"xtk{k}",
                                     name=f"xtk{k}") for k in range(6)]
                off, _, M = self.blob_off["xtk"]
                for k in range(6):
                    nc.sync.dma_start(
                        xtk[k][:],
                        self.d_blob[:, off + k * M:off + (k + 1) * M])

                # ==== phase 0: ptk = Wm^T xtk + bm (kept cols only)
                pt_cm = tc.tile_pool(name="ptp", bufs=1)
                pt_pool = pt_cm.__enter__()
                ptw = pt_pool.tile([128, 6 * KEPT], BF16, tag="ptw",
                                   name="ptw")
                ptk4 = ptw[:].rearrange("p (m t n) -> p m t n", m=6, t=C)
                wm_cm = tc.tile_pool(name="wmp", bufs=1)
                wm_pool = wm_cm.__enter__()
                wm = self.load_weight_tiles(wm_pool, "Wm")
                with tc.tile_pool(name="ps0", bufs=1, space="PSUM") as pp0:
                    for m in range(6):
                        for base in range(0, KEPT, CH):
                            n = min(CH, KEPT - base)
                            ps = pp0.tile([128, CH], F32, tag="p0",
                                          name=f"p0m{m}b{base}", bufs=2)
                            for k in range(6):
                                nc.tensor.matmul(
                                    ps[:, 0:n],
                                    wm[k][:, m * 128:(m + 1) * 128],
                                    xtk[k][:, base:base + n],
                                    start=(k == 0), stop=(k == 5))
                            pb = m * KEPT + base
                            if m % 2 == 0:
                                nc.scalar.activation(
                                    ptw[:, pb:pb + n], ps[:, 0:n],
                                    AF.Identity, bias=self.vcol("bm", m))
                            else:
                                nc.vector.tensor_scalar_add(
                                    ptw[:, pb:pb + n], ps[:, 0:n],
                                    self.vcol("bm", m))

                wm_cm.__exit__(None, None, None)

                # ==== phase 1: recurrence (standalone)
                um_cm = tc.tile_pool(name="ump", bufs=1)
                um_pool = um_cm.__enter__()
                um = self.load_weight_tiles(um_pool, "Um")
                id_off = self.blob_off["id"][0]
                id_sb = um_pool.tile([128, 128], BF16, name="id_sb")
                nc.sync.dma_start(id_sb[:],
                                  self.d_blob[:, id_off:id_off + 128])
                stp_cm = tc.tile_pool(name="stp", bufs=1)
                stp_pool = stp_cm.__enter__()
                rps_cm = tc.tile_pool(name="recps", bufs=1, space="PSUM")
                self.rps = rps_cm.__enter__()
                stpv = [stp_pool.tile([128, 6 * NC], BF16, tag=f"stp{i}",
                                      name=f"stp{i}")[:].rearrange(
                            "p (m n) -> p m n", m=6)
                        for i in range(2)]
                memc4 = memw[:].rearrange("p (m t n) -> p m t n", m=6, t=C)
                for _ in self.recurrence_q(ptk4, ptw[:], memc4, um,
                                           id_sb, zcol, stpv):
                    pass
                rps_cm.__exit__(None, None, None)
                stp_cm.__exit__(None, None, None)
                um_cm.__exit__(None, None, None)
                pt_cm.__exit__(None, None, None)

                # long-lived MLP PSUM pools (phases 2-5)
                mlp_ps = tc.tile_pool(name="mps", bufs=1, space="PSUM")
                self.pp = mlp_ps.__enter__()
                mlp_sp = tc.tile_pool(name="msp", bufs=1, space="PSUM")
                self.sp = mlp_sp.__enter__()

                # ==== phase 2: encode(x) || encode(mem), half-width streams
                allc = list(range(NCH))
                xt_chs = {c: [xtk[k][:, c * CH:(c + 1) * CH]
                              for k in range(6)] for c in allc}
                lat_ov = {c: latx[:, c * CH:(c + 1) * CH] for c in allc}
                latm_ov = {c: latm[:, c * CH:(c + 1) * CH] for c in allc}
                memv = memw[:].rearrange("p (m tn) -> p m tn", m=6)
                mem_chs = {c: [memv[:, k, c * CH:(c + 1) * CH]
                               for k in range(6)] for c in allc}
                h0 = max(1, NCH // 2)
                import os as _os
                wdec_cm = tc.tile_pool(name="wdec", bufs=1)
                wdec_pool = wdec_cm.__enter__()
                self.dec_pool = wdec_pool
                halves = [allc[:h0], allc[h0:]] if NCH > 1 else [allc]
                lrows = [wdec_pool.tile([1, 2 * len(csl) * CH], F32,
                                        tag=f"lrow{h}", name=f"lrow{h}")
                         for h, csl in enumerate(halves)]
                wd = [self.load_weight_tiles(wdec_pool, k)
                      for k in ("Wd1", "Wd2", "Wd3")]
                skip_enc = _os.environ.get("SKIP_ENC") == "1"
                skip_dec = _os.environ.get("SKIP_DEC") == "1"
                if skip_enc:
                    nc.vector.memset(latx[:], 0.1)
                    nc.vector.memset(latm[:], 0.1)
                if skip_dec:
                    for lr in lrows:
                        nc.vector.memset(lr[:], 0.1)
                # three 2-stream groups; decode half h hides under the
                # opposite-tag encode half that no longer needs its tags:
                #   [x-a(A) || m-a(B)] -> [x-b(A) || dec-a(B)]
                #   -> [m-b(B) || dec-b(A)]
                gxa = (self.encode_q("x", xt_chs, lat_ov, BF16, allc[:h0])
                       if not skip_enc else None)
                gma = (self.encode_q("m", mem_chs, latm_ov, BF16, allc[:h0])
                       if not skip_enc else None)
                gxb = (self.encode_q("x", xt_chs, lat_ov, BF16, allc[h0:])
                       if not skip_enc and NCH > 1 else None)
                gmb = (self.encode_q("m", mem_chs, latm_ov, BF16,
                                     allc[h0:])
                       if not skip_enc and NCH > 1 else None)
                gda = (self.decode_q("d2", latx, xtk, wd, lrows[0],
                                     halves[0]) if not skip_dec else None)
                gdb = (self.decode_q("d", latx, xtk, wd, lrows[1],
                                     halves[1])
                       if not skip_dec and NCH > 1 else None)
                for grp in ([gxa, gma], [gxb, gda], [gmb, gdb]):
                    grp = [g for g in grp if g is not None]
                    if grp:
                        _interleave(grp, [1] * len(grp))

                # ==== phase 5: ctx + combine + output
                for h, csl in enumerate(halves):
                    for _ in self.ctx_q(latx, latm, lrows[h], csl):
                        pass

                wdec_cm.__exit__(None, None, None)
                mlp_sp.__exit__(None, None, None)
                mlp_ps.__exit__(None, None, None)
                xtk_cm.__exit__(None, None, None)
        nc.compile()
        return nc


def _chain(*gens):
    for g in gens:
        if g is not None:
            yield from g


def _interleave(gens, weights):
    gens = list(gens)
    weights = list(weights)
    while gens:
        for i in range(len(gens) - 1, -1, -1):
            try:
                for _ in range(weights[i]):
                    next(gens[i])
            except StopIteration:
                del gens[i]
                del weights[i]


# ---------------------------------------------------------------- runner

_CACHE = {}


def _get_built(S, BL):
    key = (S, BL)
    if key not in _CACHE:
        kb = _KB(S, BL)
        kb.build()
        _CACHE[key] = kb
    return _CACHE[key]


def _host_inputs(kb, inputs):
    S, BL, C, W, T, NC = kb.S, kb.BL, kb.C, kb.W, kb.T, kb.NC
    w = {k: np.asarray(v, np.float32) for k, v in inputs.items()}
    Wvo = w["Wv"] @ w["Wo"]
    bvo = w["bv"] @ w["Wo"] + w["bo"]
    wd = dict(w)
    # fold the (linear) self-attn projection into W2: a@W2 = h@(Wvo@W2)
    wd["W2"] = Wvo @ w["W2"]
    wd["b2"] = bvo @ w["W2"] + w["b2"]
    z0 = _encode_np(np.zeros((1, D), np.float32), wd)[0]

    vecs = _pack_cols(w["b1"], w["g1"], w["be1"],
                      wd["b2"], w["g2"], w["be2"],
                      w["b3"], w["g3"], w["be3"],
                      w["b4"], w["g4"], w["be4"],
                      w["b5"], w["g5"], w["be5"],
                      w["bd1"], w["gd1"], w["bed1"],
                      w["bd2"], w["gd2"], w["bed2"],
                      w["bd3"], w["bm"], z0,
                      np.full(128, LN_EPS, np.float32))

    def b16(x):
        return np.ascontiguousarray(x.astype(ml_dtypes.bfloat16))

    wd["id"] = np.eye(128, dtype=np.float32)
    wd["vecs"] = vecs
    blob_off, nblob = kb.blob_layout()
    wblob = np.zeros((128, nblob), ml_dtypes.bfloat16)
    for name, (off, ntiles, M) in blob_off.items():
        if name == "xtk":
            continue
        wsrc = np.asarray(wd[name], np.float32)
        for k in range(ntiles):
            wblob[:, off + k * M:off + (k + 1) * M] = b16(
                wsrc[k * 128:(k + 1) * 128, :])

    # kept tokens in chunk-step order: col(t', cid, b) = t'*NC + cid*BL + b
    seqs = np.asarray(inputs["sequences"], np.float32)
    ncid = S // C
    in_maps = []
    xtk_off = blob_off["xtk"]
    for core in range(NCORES):
        xs = seqs[core * BL:(core + 1) * BL, :S, :]       # [BL,S,D]
        g = xs.reshape(BL, ncid, C, D)                    # [BL,cid,t',D]
        g = np.transpose(g, (2, 1, 0, 3))                 # [t',cid,BL,D]
        gt16 = b16(g.reshape(kb.KEPT, D).T)               # [D, KEPT]
        blob = wblob.copy()
        off, _, M = xtk_off
        for k in range(6):
            blob[:, off + k * M:off + (k + 1) * M] = \
                gt16[k * 128:(k + 1) * 128, :]
        in_maps.append(dict(blob16=blob))
    return in_maps


def _l2_term(inputs):
    names = ["W1", "b1", "g1", "be1", "Wv", "bv", "Wo", "bo", "W2", "b2",
             "g2", "be2", "W3", "b3", "g3", "be3", "W4", "b4", "g4", "be4",
             "W5", "b5", "g5", "be5", "Wd1", "bd1", "gd1", "bed1", "Wd2",
             "bd2", "gd2", "bed2", "Wd3", "bd3", "Wm", "Um", "bm"]
    l2 = sum(np.linalg.norm(np.asarray(inputs[n], np.float64))
             for n in names)
    return float(np.clip(l2, 0.0, 10.0))


def _combine(kb, res, inputs):
    tok = np.concatenate([res.results[c]["tok_loss"].reshape(-1)
                          for c in range(NCORES)])
    l2 = _l2_term(inputs)
    per_tok = np.clip(tok.astype(np.float64) + 1e-4 * l2, 0.0, 100.0)
    nb = kb.BL * NCORES
    return np.float32(per_tok.sum() / nb)


def kernel(**inputs):
    seqs = np.asarray(inputs["sequences"])
    S = seqs.shape[1]
    BL = seqs.shape[0] // NCORES
    kb = _get_built(S, BL)
    in_maps = _host_inputs(kb, inputs)
    res = run_bass_kernel_spmd(kb.nc, in_maps, list(range(NCORES)))
    return _combine(kb, res, inputs)
